# revision 14
# baseline (speedup 1.0000x reference)
"""GQA attention block on 8 Trainium2 NeuronCores (tensor-parallel by heads).

Shapes (hardcoded): x [1, 2048, 4096], W_qkv [4096, 6144] (32 Q + 8 K + 8 V
heads, head_dim 128), W_out [4096, 4096]. Partial interleaved RoPE over the
first 64 head dims, full (non-causal) softmax.

Sharding: core g owns KV head g and Q heads 4g..4g+3 (W_qkv columns
[K, V, Q0..Q3] = 768, W_out rows 512g..512(g+1)). Each core computes a
partial output projection in bf16; the host sums the 8 partials in f32.

Host-side preprocessing:
  - x passed pre-transposed as x^T [4096, 2048] in bf16, tiled [128, 32, 512].
  - RoPE pairs de-interleaved by permuting W_qkv columns per rot head
    (Q and K permuted identically => scores unchanged); 1/sqrt(128) folded
    into W_q (RoPE is a rotation, commutes with scaling).
  - cos/sin tables precomputed: ctab [128, S] = [cos; cos; ones],
    stab [64, S] = [-sin; sin].

Device pipeline per core (matmuls bf16, f32 PSUM accumulate):
  Phase 1: QKV^T = W^T x^T streamed over 4 s-slices; RoPE finish of the
    previous (slice, head-tile) group is deferred behind the next group's
    matmul stream so the PE never waits on the scalar-engine PSUM copy.
  Phase 2 (per (head, q-slice) unit, software-pipelined over 3 iterations):
    scores^T matmuls write bf16 PSUM (half bank per 128-k chunk); exp runs
    as 4 FD=2048 activations; the softmax denominator is a DVE pairwise
    tree over the 16 exp chunks + a GPSIMD partition_all_reduce (so no
    ones-matmul, no [1,512] reciprocal, no broadcast matmul); A@V
    accumulates f32; normalization is a single DVE divide fused into the
    PSUM->SBUF evacuation of O^T.
  Phase 3: output projection with bf16 SBUF/DMA.
"""

import numpy as np
import ml_dtypes

import concourse.bass as bass
import concourse.bass_isa as bass_isa
import concourse.mybir as mybir
import concourse.tile as tile
from concourse import library_config
from concourse.bass_utils import run_bass_kernel_spmd
from concourse.masks import make_identity

BF16 = ml_dtypes.bfloat16

P = 128
S = 2048
HIDDEN = 4096
HCH = HIDDEN // P          # 32 contraction chunks
SQ = 512                   # s-slice width
NQ = S // SQ               # 4 s-slices
AT = 6                     # a-tiles per core: 0=K, 1=V, 2..5=Q0..Q3
QH = 4                     # q heads per core
ROT = 64
N_CORES = 8
NU = QH * NQ               # 16 attention units

f32 = mybir.dt.float32
bf16 = mybir.dt.bfloat16
MULT = mybir.AluOpType.mult
ADD = mybir.AluOpType.add
EXP = mybir.ActivationFunctionType.Exp


def _phase1(tc, d, ident, qkvT, v_sb):
    nc = tc.nc
    with (
        tc.tile_pool(name="p1const", bufs=1) as cp,
        tc.tile_pool(name="xq", bufs=2) as xqp,
        tc.tile_pool(name="wq", bufs=1) as wqp,
        tc.tile_pool(name="work1", bufs=2) as workp,
        tc.tile_pool(name="ps1", bufs=1, space="PSUM") as psp,
    ):
        # DMA order: first-needed first so the PE can start ASAP.
        wqt = [wqp.tile([P, HCH, P], bf16, tag=f"w{a}", name=f"w{a}")
               for a in range(AT)]
        nc.sync.dma_start(out=wqt[0][:], in_=d["wq"][0])

        def load_xq(q, parts=2):
            t = xqp.tile([P, HCH, SQ], bf16, tag="xq", name=f"xq{q}")
            step = HCH // parts
            for b in range(parts):
                nc.sync.dma_start(out=t[:, b * step:(b + 1) * step, :],
                                  in_=d["xq"][q, :, b * step:(b + 1) * step, :])
            return t

        # xq[0] in quarters so the first matmul group starts as soon as the
        # first contraction chunks land; constants only gate the first RoPE
        # finish, which runs an entire matmul group later.
        xcur = load_xq(0, parts=4)
        ctab = cp.tile([P, S], f32, tag="ctab")
        nc.sync.dma_start(out=ctab[:], in_=d["ctab"][:])
        stab = cp.tile([ROT, S], f32, tag="stab")
        nc.sync.dma_start(out=stab[:], in_=d["stab"][:])
        perm = cp.tile([ROT, ROT], f32, tag="perm")
        nc.sync.dma_start(out=perm[:], in_=d["perm"][:])
        for a in range(1, AT):
            nc.sync.dma_start(out=wqt[a][:], in_=d["wq"][a])

        def finish(q, a, ps):
            sl = slice(q * SQ, (q + 1) * SQ)
            if a == 1:
                # V: cast to bf16, PE-transpose to V[k,d] chunks
                vst = workp.tile([P, SQ], bf16, tag="vst", bufs=2)
                nc.scalar.copy(vst[:], ps[:])
                pst = psp.tile([P, 4, P], bf16, tag="vt", bufs=2)
                for i in range(4):
                    nc.tensor.transpose(pst[:, i, :], vst[:, i * P:(i + 1) * P],
                                        ident[:])
                nc.vector.tensor_copy(v_sb[:, 4 * q:4 * q + 4, :], pst[:])
            else:
                qi = 0 if a == 0 else a - 1
                stage = workp.tile([P, SQ], f32, tag="stage", bufs=2)
                nc.scalar.copy(stage[:], ps[:])
                swp = psp.tile([ROT, SQ], f32, tag="rope", bufs=2)
                nc.tensor.matmul(swp[:], perm[:], stage[0:ROT, :],
                                 start=True, stop=True)
                tmp = workp.tile([ROT, SQ], f32, tag="tmp", bufs=2)
                nc.vector.tensor_tensor(tmp[:], swp[:], stab[:, sl], op=MULT)
                rot = workp.tile([P, SQ], f32, tag="rot", bufs=2)
                nc.vector.tensor_tensor(rot[:], stage[:], ctab[:, sl], op=MULT)
                nc.vector.tensor_tensor(rot[0:ROT, :], rot[0:ROT, :],
                                        tmp[:], op=ADD)
                nc.vector.tensor_copy(qkvT[qi][:, sl], rot[:])

        prev = None
        for q in range(NQ):
            if q + 1 < NQ:
                xnext = load_xq(q + 1)
            for a in range(AT):
                ps = psp.tile([P, SQ], f32, tag="acc", bufs=2)
                for c in range(HCH):
                    nc.tensor.matmul(ps[:], wqt[a][:, c, :], xcur[:, c, :],
                                     start=(c == 0), stop=(c == HCH - 1))
                if prev is not None:
                    finish(*prev)
                prev = (q, a, ps)
            if q + 1 < NQ:
                xcur = xnext
        finish(*prev)


def _phase2(tc, d, ones128, onesk1, qkvT, v_sb, ot_sb):
    """Per unit (head, q-slice), 3-deep software pipeline:
      iter k   : ST+exp of unit k; DVE tree + A@V + colsum-matmul of k-1;
                 recip/bcast/lb-copy/normalize of k-2.
    The single-partition reciprocal (~3.3us DVE) never blocks the PE: the
    bcast matmul that consumes it runs mid-NEXT-iteration."""
    nc = tc.nc
    with (
        tc.tile_pool(name="pt", bufs=1) as ptp,
        tc.tile_pool(name="sum2", bufs=1) as sump,
        tc.tile_pool(name="ps2", bufs=1, space="PSUM") as psp,
    ):
        kT = qkvT[0]
        units = [(h, j) for j in range(NQ) for h in range(QH)]
        pts, ops, lbs, accs, rrs, lps = {}, {}, {}, {}, {}, {}

        def st_group(k, g):
            # 2 k-chunks per group: [128, 2, 512] f32 = 2 PSUM banks,
            # exp drains both with one FD=1024 activation.
            h, j = units[k]
            if g == 0:
                pts[k] = ptp.tile([P, 16, SQ], bf16, tag="pt", bufs=2,
                                  name=f"pt{k}")
            stg = psp.tile([P, 2, SQ], f32, tag="stg", bufs=2, name="stg")
            for i in range(2):
                c = 2 * g + i
                nc.tensor.matmul(stg[:, i, :], kT[:, c * P:(c + 1) * P],
                                 qkvT[1 + h][:, j * SQ:(j + 1) * SQ],
                                 start=True, stop=True)
            nc.scalar.activation(pts[k][:, 2 * g:2 * g + 2, :], stg[:], EXP)

        def tree(k):
            pt = pts[k]
            acc8 = sump.tile([P, 8, SQ], bf16, tag="acc8", bufs=1)
            nc.vector.tensor_tensor(acc8[:], pt[:, 0:8, :], pt[:, 8:16, :],
                                    op=ADD)
            acc4 = sump.tile([P, 4, SQ], bf16, tag="acc4", bufs=1)
            nc.vector.tensor_tensor(acc4[:], acc8[:, 0:4, :], acc8[:, 4:8, :],
                                    op=ADD)
            # last two levels on the (otherwise idle) GPSIMD engine to
            # keep DVE below the scalar-engine pace
            acc2 = sump.tile([P, 2, SQ], bf16, tag="acc2", bufs=1)
            nc.gpsimd.tensor_tensor(acc2[:], acc4[:, 0:2, :], acc4[:, 2:4, :],
                                    op=ADD)
            accs[k] = sump.tile([P, SQ], bf16, tag="accS", bufs=2,
                                name=f"accS{k}")
            nc.gpsimd.tensor_tensor(accs[k][:], acc2[:, 0, :], acc2[:, 1, :],
                                    op=ADD)

        def colsum(k):
            # single ones-matmul over the tree result -> l in lp[0:1, :]
            lps[k] = psp.tile([P, SQ], f32, tag="lp", bufs=2, name="lp")
            nc.tensor.matmul(lps[k][0:1, :], ones128[:], accs[k][:],
                             start=True, stop=True)

        def recip(k):
            rrs[k] = sump.tile([1, SQ], f32, tag="rr", bufs=2, name=f"rr{k}")
            nc.vector.reciprocal(rrs[k][:], lps[k][0:1, :])

        def bcast(k):
            # broadcast 1/l to 128 partitions via K=1 matmul, then to SBUF
            nc.tensor.matmul(lps[k][:], onesk1[:], rrs[k][:],
                             start=True, stop=True)

        def lcopy(k):
            lbs[k] = sump.tile([P, SQ], f32, tag="lb", bufs=2, name=f"lb{k}")
            nc.scalar.copy(lbs[k][:], lps[k][:])

        def av_chunks(k, c0, c1):
            if c0 == 0:
                ops[k] = psp.tile([P, SQ], f32, tag="op", bufs=2, name=f"op{k}")
            for c in range(c0, c1):
                nc.tensor.matmul(ops[k][:], v_sb[:, c, :], pts[k][:, c, :],
                                 start=(c == 0), stop=(c == 15))

        def norm(k):
            h, j = units[k]
            nc.vector.tensor_tensor(ot_sb[:, h, j * SQ:(j + 1) * SQ],
                                    ops[k][:], lbs[k][:], op=MULT)
            del pts[k], ops[k], lbs[k], accs[k], rrs[k], lps[k]

        for k in range(NU + 3):
            u0, u1, u2 = k, k - 1, k - 2
            live0 = u0 < NU
            live1 = 0 <= u1 < NU
            live2 = 0 <= u2 < NU
            if live2:
                recip(u2)
            if live0:
                st_group(u0, 0)
                st_group(u0, 1)
            if live1:
                tree(u1)
            # interleave the previous unit's 16 A@V chunk-matmuls between
            # ST groups so the PE fills the scalar-engine drain waits
            for g in range(2, 8):
                if live1:
                    av_chunks(u1, 2 * (g - 2), 2 * (g - 1))
                if live0:
                    st_group(u0, g)
                if g == 3 and live2:
                    bcast(u2)
                    lcopy(u2)
            if live1:
                av_chunks(u1, 12, 16)
                colsum(u1)
            if live2:
                norm(u2)


def _phase3(tc, d, woutt, ot_sb):
    nc = tc.nc
    with (
        tc.tile_pool(name="y", bufs=1) as yp,
        tc.tile_pool(name="ps3", bufs=1, space="PSUM") as psp,
    ):
        for i in range(S // P):
            ysb = yp.tile([P, HIDDEN], bf16, tag="ysb", bufs=2)
            for n in range(HIDDEN // SQ):
                yps = psp.tile([P, SQ], f32, tag="yps", bufs=4)
                for hc in range(QH):
                    nc.tensor.matmul(yps[:], ot_sb[:, hc, i * P:(i + 1) * P],
                                     woutt[hc][:, n * SQ:(n + 1) * SQ],
                                     start=(hc == 0), stop=(hc == QH - 1))
                nc.vector.tensor_copy(ysb[:, n * SQ:(n + 1) * SQ], yps[:])
            nc.sync.dma_start(out=d["y"][i * P:(i + 1) * P, :], in_=ysb[:])


def _emit(tc, d):
    nc = tc.nc
    with (
        tc.tile_pool(name="const", bufs=1) as constp,
        tc.tile_pool(name="persist", bufs=1) as pp,
    ):
        ident = constp.tile([P, P], bf16, tag="ident")
        make_identity(nc, ident[:])
        ones128 = constp.tile([P, 1], bf16, tag="ones128")
        nc.gpsimd.memset(ones128[:], 1.0)
        onesk1 = constp.tile([1, P], f32, tag="onesk1")
        nc.gpsimd.memset(onesk1[:], 1.0)

        qkvT = [pp.tile([P, S], bf16, tag=f"qkv{i}", name=f"qkv{i}")
                for i in range(5)]
        v_sb = pp.tile([P, 16, P], bf16, tag="v")      # V[k,d] in 16 k-chunks
        ot_sb = pp.tile([P, QH, S], bf16, tag="ot")    # O^T per head [d, s]

        _phase1(tc, d, ident, qkvT, v_sb)

        with tc.tile_pool(name="wo", bufs=1) as wop:
            woutt = [wop.tile([P, HIDDEN], bf16, tag=f"wo{c}", name=f"wo{c}")
                     for c in range(QH)]
            for c in range(QH):
                nc.sync.dma_start(out=woutt[c][:],
                                  in_=d["wout"][c * P:(c + 1) * P, :])
            _phase2(tc, d, ones128, onesk1, qkvT, v_sb, ot_sb)
            _phase3(tc, d, woutt, ot_sb)


def _legalize_waits(nc):
    """This toolchain's codegen accepts at most ONE sync wait per
    instruction; hoist extra waits onto single-wait Drain clones inserted
    just before the instruction on the same engine."""
    import copy
    f = nc.m.functions[0]
    templates = {}
    for blk in f.blocks:
        for inst in blk.instructions:
            if type(inst).__name__ == "InstDrain":
                templates.setdefault(str(inst.engine), inst)
    anyt = next(iter(templates.values()))
    SI = type(next(i for b in f.blocks for i in b.instructions
                   if i.sync_info).sync_info)
    k = 0
    for blk in f.blocks:
        newl = []
        for inst in blk.instructions:
            si = inst.sync_info
            if si and len(si.on_wait) > 1:
                for w in si.on_wait:
                    dcl = copy.deepcopy(templates.get(str(inst.engine), anyt))
                    dcl.engine = inst.engine
                    dcl.name = f"{inst.name}w{k}"; k += 1
                    dcl.sync_info = SI(on_wait=[w], on_update=[])
                    newl.append(dcl)
                inst.sync_info = SI(on_wait=[], on_update=list(si.on_update))
            newl.append(inst)
        try:
            blk.instructions[:] = newl
        except Exception:
            blk.instructions = newl


def build():
    nc = bass.Bass()
    d = {
        "xq": nc.dram_tensor("xq", [NQ, P, HCH, SQ], bf16, kind="ExternalInput"),
        "wq": nc.dram_tensor("wq", [AT, P, HCH, P], bf16, kind="ExternalInput"),
        "wout": nc.dram_tensor("wout", [QH * P, HIDDEN], bf16,
                               kind="ExternalInput"),
        "ctab": nc.dram_tensor("ctab", [P, S], f32, kind="ExternalInput"),
        "stab": nc.dram_tensor("stab", [ROT, S], f32, kind="ExternalInput"),
        "perm": nc.dram_tensor("perm", [ROT, ROT], f32, kind="ExternalInput"),
        "y": nc.dram_tensor("y", [S, HIDDEN], bf16, kind="ExternalOutput"),
    }
    with tile.TileContext(nc) as tc:
        _emit(tc, d)
    _legalize_waits(nc)
    return nc


_NC_CACHE = {}


def _get_nc():
    if "nc" not in _NC_CACHE:
        _NC_CACHE["nc"] = build()
    return _NC_CACHE["nc"]


def make_in_maps(x, W_qkv, W_out):
    xT = np.ascontiguousarray(x.reshape(S, HIDDEN).T)
    # [4096, 2048] -> (c p) (q s) -> [q, p, c, s]
    xq = np.ascontiguousarray(
        xT.reshape(HCH, P, NQ, SQ).transpose(2, 1, 0, 3)).astype(BF16)

    perm = np.concatenate([np.arange(0, ROT, 2), np.arange(1, ROT, 2),
                           np.arange(ROT, P)])
    scale = np.float32(P ** -0.5)

    inv = (10000.0 ** (-np.arange(32) / 32.0)).astype(np.float64)
    fr = np.arange(S, dtype=np.float64)[:, None] * inv[None, :]
    cos = np.cos(fr).T.astype(np.float32)   # [32, S]
    sin = np.sin(fr).T.astype(np.float32)
    ctab = np.concatenate([cos, cos, np.ones((ROT, S), np.float32)], axis=0)
    stab = np.concatenate([-sin, sin], axis=0)
    # swap matrix: swp[m, :] = stage[(m+32) % 64, :]
    permm = np.zeros((ROT, ROT), np.float32)
    for m in range(ROT):
        permm[(m + 32) % ROT, m] = 1.0

    in_maps = []
    for g in range(N_CORES):
        kcol = W_qkv[:, (32 + g) * P:(33 + g) * P][:, perm]
        vcol = W_qkv[:, (40 + g) * P:(41 + g) * P]
        qcols = [W_qkv[:, (4 * g + i) * P:(4 * g + i + 1) * P][:, perm] * scale
                 for i in range(QH)]
        wc = np.stack([kcol, vcol] + qcols, axis=0)          # [6, 4096, 128]
        wq = np.ascontiguousarray(
            wc.reshape(AT, HCH, P, P).transpose(0, 2, 1, 3)).astype(BF16)
        wout = np.ascontiguousarray(
            W_out[g * QH * P:(g + 1) * QH * P, :]).astype(BF16)
        in_maps.append({"xq": xq, "wq": wq, "wout": wout,
                        "ctab": ctab, "stab": stab, "perm": permm})
    return in_maps


def run(x, W_qkv, W_out, trace=False):
    nc = _get_nc()
    in_maps = make_in_maps(np.asarray(x, dtype=np.float32),
                           np.asarray(W_qkv, dtype=np.float32),
                           np.asarray(W_out, dtype=np.float32))
    res = run_bass_kernel_spmd(nc, in_maps, list(range(N_CORES)), trace=trace)
    y = np.zeros((S, HIDDEN), np.float32)
    for om in res.results:
        y += np.asarray(om["y"], dtype=np.float32)
    return y.reshape(1, S, HIDDEN), res


def kernel(x, W_qkv, W_out):
    y, _ = run(x, W_qkv, W_out)
    return y


# revision 19
# speedup vs baseline: 1.0586x; 1.0586x over previous
"""GQA attention block on 8 Trainium2 NeuronCores (tensor-parallel by heads).

Shapes (hardcoded): x [1, 2048, 4096], W_qkv [4096, 6144] (32 Q + 8 K + 8 V
heads, head_dim 128), W_out [4096, 4096]. Partial interleaved RoPE over the
first 64 head dims, full (non-causal) softmax.

Sharding: core g owns KV head g and Q heads 4g..4g+3 (W_qkv columns
[K, V, Q0..Q3] = 768, W_out rows 512g..512(g+1)). Each core computes a
partial output projection in bf16; the host sums the 8 partials in f32.

Host-side preprocessing:
  - x passed pre-transposed as x^T [4096, 2048] in bf16, tiled [128, 32, 512].
  - RoPE pairs de-interleaved by permuting W_qkv columns per rot head
    (Q and K permuted identically => scores unchanged); 1/sqrt(128) folded
    into W_q (RoPE is a rotation, commutes with scaling).
  - cos/sin tables precomputed: ctab [128, S] = [cos; cos; ones],
    stab [64, S] = [-sin; sin].

Device pipeline per core (matmuls bf16, f32 PSUM accumulate):
  Phase 1: QKV^T = W^T x^T streamed over 4 s-slices; RoPE finish of the
    previous (slice, head-tile) group is deferred behind the next group's
    matmul stream so the PE never waits on the scalar-engine PSUM copy.
  Phase 2 (per (head, q-slice) unit, software-pipelined over 3 iterations):
    scores^T matmuls write bf16 PSUM (half bank per 128-k chunk); exp runs
    as 4 FD=2048 activations; the softmax denominator is a DVE pairwise
    tree over the 16 exp chunks + a GPSIMD partition_all_reduce (so no
    ones-matmul, no [1,512] reciprocal, no broadcast matmul); A@V
    accumulates f32; normalization is a single DVE divide fused into the
    PSUM->SBUF evacuation of O^T.
  Phase 3: output projection with bf16 SBUF/DMA.
"""

import numpy as np
import ml_dtypes

import concourse.bass as bass
import concourse.bass_isa as bass_isa
import concourse.mybir as mybir
import concourse.tile as tile
from concourse import library_config
from concourse.bass_utils import run_bass_kernel_spmd
from concourse.masks import make_identity

BF16 = ml_dtypes.bfloat16

P = 128
S = 2048
HIDDEN = 4096
HCH = HIDDEN // P          # 32 contraction chunks
SQ = 512                   # s-slice width
NQ = S // SQ               # 4 s-slices
AT = 6                     # a-tiles per core: 0=K, 1=V, 2..5=Q0..Q3
QH = 4                     # q heads per core
ROT = 64
N_CORES = 8
NU = QH * NQ               # 16 attention units

f32 = mybir.dt.float32
bf16 = mybir.dt.bfloat16
MULT = mybir.AluOpType.mult
ADD = mybir.AluOpType.add
EXP = mybir.ActivationFunctionType.Exp


def _phase1(tc, d, ident, qkvT, v_sb):
    nc = tc.nc
    with (
        tc.tile_pool(name="p1const", bufs=1) as cp,
        tc.tile_pool(name="xq", bufs=2) as xqp,
        tc.tile_pool(name="wq", bufs=1) as wqp,
        tc.tile_pool(name="work1", bufs=2) as workp,
        tc.tile_pool(name="ps1", bufs=1, space="PSUM") as psp,
    ):
        # DMA order: first-needed first so the PE can start ASAP.
        wqt = [wqp.tile([P, HCH, P], bf16, tag=f"w{a}", name=f"w{a}")
               for a in range(AT)]
        nc.sync.dma_start(out=wqt[0][:], in_=d["wq"][0])

        def load_xq(q, parts=2):
            t = xqp.tile([P, HCH, SQ], bf16, tag="xq", name=f"xq{q}")
            step = HCH // parts
            for b in range(parts):
                nc.sync.dma_start(out=t[:, b * step:(b + 1) * step, :],
                                  in_=d["xq"][q, :, b * step:(b + 1) * step, :])
            return t

        # xq[0] in quarters so the first matmul group starts as soon as the
        # first contraction chunks land; constants only gate the first RoPE
        # finish, which runs an entire matmul group later.
        xcur = load_xq(0, parts=4)
        ctab = cp.tile([P, S], f32, tag="ctab")
        nc.sync.dma_start(out=ctab[:], in_=d["ctab"][:])
        stab = cp.tile([ROT, S], f32, tag="stab")
        nc.sync.dma_start(out=stab[:], in_=d["stab"][:])
        perm = cp.tile([ROT, ROT], f32, tag="perm")
        nc.sync.dma_start(out=perm[:], in_=d["perm"][:])
        for a in range(1, AT):
            nc.sync.dma_start(out=wqt[a][:], in_=d["wq"][a])

        def finish(q, a, ps):
            sl = slice(q * SQ, (q + 1) * SQ)
            if a == 1:
                # V: cast to bf16, PE-transpose to V[k,d] chunks
                vst = workp.tile([P, SQ], bf16, tag="vst", bufs=2)
                nc.scalar.copy(vst[:], ps[:])
                pst = psp.tile([P, 4, P], bf16, tag="vt", bufs=2)
                for i in range(4):
                    nc.tensor.transpose(pst[:, i, :], vst[:, i * P:(i + 1) * P],
                                        ident[:])
                nc.vector.tensor_copy(v_sb[:, 4 * q:4 * q + 4, :], pst[:])
            else:
                qi = 0 if a == 0 else a - 1
                stage = workp.tile([P, SQ], f32, tag="stage", bufs=2)
                nc.scalar.copy(stage[:], ps[:])
                swp = psp.tile([ROT, SQ], f32, tag="rope", bufs=2)
                nc.tensor.matmul(swp[:], perm[:], stage[0:ROT, :],
                                 start=True, stop=True)
                tmp = workp.tile([ROT, SQ], f32, tag="tmp", bufs=2)
                nc.vector.tensor_tensor(tmp[:], swp[:], stab[:, sl], op=MULT)
                rot = workp.tile([P, SQ], f32, tag="rot", bufs=2)
                nc.vector.tensor_tensor(rot[:], stage[:], ctab[:, sl], op=MULT)
                nc.vector.tensor_tensor(rot[0:ROT, :], rot[0:ROT, :],
                                        tmp[:], op=ADD)
                nc.vector.tensor_copy(qkvT[qi][:, sl], rot[:])

        prev = None
        for q in range(NQ):
            if q + 1 < NQ:
                xnext = load_xq(q + 1)
            for a in range(AT):
                ps = psp.tile([P, SQ], f32, tag="acc", bufs=2)
                for c in range(HCH):
                    nc.tensor.matmul(ps[:], wqt[a][:, c, :], xcur[:, c, :],
                                     start=(c == 0), stop=(c == HCH - 1))
                if prev is not None:
                    finish(*prev)
                prev = (q, a, ps)
            if q + 1 < NQ:
                xcur = xnext
        finish(*prev)


def _phase2(tc, d, ones128, onesk1, qkvT, v_sb, ot_sb):
    """Per unit (head, q-slice), 3-deep software pipeline:
      iter k   : ST+exp of unit k; DVE tree + A@V + colsum-matmul of k-1;
                 recip/bcast/lb-copy/normalize of k-2.
    The single-partition reciprocal (~3.3us DVE) never blocks the PE: the
    bcast matmul that consumes it runs mid-NEXT-iteration."""
    nc = tc.nc
    with (
        tc.tile_pool(name="pt", bufs=1) as ptp,
        tc.tile_pool(name="sum2", bufs=1) as sump,
        tc.tile_pool(name="ps2", bufs=1, space="PSUM") as psp,
    ):
        kT = qkvT[0]
        units = [(h, j) for j in range(NQ) for h in range(QH)]
        pts, ops, lbs, accs, rrs, lps = {}, {}, {}, {}, {}, {}

        def st_group(k, g):
            # 2 k-chunks per group: [128, 2, 512] f32 = 2 PSUM banks,
            # exp drains both with one FD=1024 activation.
            h, j = units[k]
            if g == 0:
                pts[k] = ptp.tile([P, 16, SQ], bf16, tag="pt", bufs=2,
                                  name=f"pt{k}")
            stg = psp.tile([P, 2, SQ], f32, tag="stg", bufs=2, name="stg")
            for i in range(2):
                c = 2 * g + i
                nc.tensor.matmul(stg[:, i, :], kT[:, c * P:(c + 1) * P],
                                 qkvT[1 + h][:, j * SQ:(j + 1) * SQ],
                                 start=True, stop=True)
            nc.scalar.activation(pts[k][:, 2 * g:2 * g + 2, :], stg[:], EXP)

        def tree(k):
            pt = pts[k]
            acc8 = sump.tile([P, 8, SQ], bf16, tag="acc8", bufs=1)
            nc.vector.tensor_tensor(acc8[:], pt[:, 0:8, :], pt[:, 8:16, :],
                                    op=ADD)
            acc4 = sump.tile([P, 4, SQ], bf16, tag="acc4", bufs=1)
            nc.vector.tensor_tensor(acc4[:], acc8[:, 0:4, :], acc8[:, 4:8, :],
                                    op=ADD)
            acc2 = sump.tile([P, 2, SQ], bf16, tag="acc2", bufs=1)
            nc.vector.tensor_tensor(acc2[:], acc4[:, 0:2, :], acc4[:, 2:4, :],
                                    op=ADD)
            accs[k] = sump.tile([P, SQ], bf16, tag="accS", bufs=2,
                                name=f"accS{k}")
            nc.vector.tensor_tensor(accs[k][:], acc2[:, 0, :], acc2[:, 1, :],
                                    op=ADD)

        def colsum(k):
            # single ones-matmul over the tree result -> l in lp[0:1, :]
            lps[k] = psp.tile([P, SQ], f32, tag="lp", bufs=2, name="lp")
            nc.tensor.matmul(lps[k][0:1, :], ones128[:], accs[k][:],
                             start=True, stop=True)

        def recip(k):
            # bf16 output so the broadcast matmul is single-pass bf16
            rrs[k] = sump.tile([1, SQ], bf16, tag="rr", bufs=2, name=f"rr{k}")
            with nc.allow_low_precision(reason="1/l in bf16: 0.4% rel, "
                                        "within the softmax error budget"):
                nc.vector.reciprocal(rrs[k][:], lps[k][0:1, :])

        def bcast(k):
            # broadcast 1/l to 128 partitions via K=1 matmul, then to SBUF
            nc.tensor.matmul(lps[k][:], onesk1[:], rrs[k][:],
                             start=True, stop=True)

        def lcopy(k):
            lbs[k] = sump.tile([P, SQ], f32, tag="lb", bufs=2, name=f"lb{k}")
            nc.scalar.copy(lbs[k][:], lps[k][:])

        def av_chunks(k, c0, c1):
            if c0 == 0:
                ops[k] = psp.tile([P, SQ], f32, tag="op", bufs=2, name=f"op{k}")
            for c in range(c0, c1):
                nc.tensor.matmul(ops[k][:], v_sb[:, c, :], pts[k][:, c, :],
                                 start=(c == 0), stop=(c == 15))

        def norm(k):
            h, j = units[k]
            nc.vector.tensor_tensor(ot_sb[:, h, j * SQ:(j + 1) * SQ],
                                    ops[k][:], lbs[k][:], op=MULT)
            del pts[k], ops[k], lbs[k], accs[k], rrs[k], lps[k]

        for k in range(NU + 3):
            u0, u1, u2 = k, k - 1, k - 2
            live0 = u0 < NU
            live1 = 0 <= u1 < NU
            live2 = 0 <= u2 < NU
            if live0:
                st_group(u0, 0)
                st_group(u0, 1)
            if live1:
                tree(u1)
            # interleave the previous unit's 16 A@V chunk-matmuls between
            # ST groups so the PE fills the scalar-engine drain waits;
            # colsum lands mid-iter (tree done by then), its reciprocal
            # finishes this iter, and the dependent bcast runs early the
            # NEXT iter so the PE never waits on the reciprocal.
            for g in range(2, 8):
                if live1:
                    av_chunks(u1, 2 * (g - 2), 2 * (g - 1))
                if live0:
                    st_group(u0, g)
                if g == 3 and live2:
                    bcast(u2)
                    lcopy(u2)
                if g == 4 and live1:
                    colsum(u1)
                    recip(u1)
            if live1:
                av_chunks(u1, 12, 16)
            if live2:
                norm(u2)


def _phase3(tc, d, woutt, ot_sb):
    nc = tc.nc
    with (
        tc.tile_pool(name="y", bufs=1) as yp,
        tc.tile_pool(name="ps3", bufs=1, space="PSUM") as psp,
    ):
        for i in range(S // P):
            ysb = yp.tile([P, HIDDEN], bf16, tag="ysb", bufs=2)
            for n in range(HIDDEN // SQ):
                yps = psp.tile([P, SQ], f32, tag="yps", bufs=4)
                for hc in range(QH):
                    nc.tensor.matmul(yps[:], ot_sb[:, hc, i * P:(i + 1) * P],
                                     woutt[hc][:, n * SQ:(n + 1) * SQ],
                                     start=(hc == 0), stop=(hc == QH - 1))
                nc.vector.tensor_copy(ysb[:, n * SQ:(n + 1) * SQ], yps[:])
            nc.sync.dma_start(out=d["y"][i * P:(i + 1) * P, :], in_=ysb[:])


def _emit(tc, d):
    nc = tc.nc
    with (
        tc.tile_pool(name="const", bufs=1) as constp,
        tc.tile_pool(name="persist", bufs=1) as pp,
    ):
        ident = constp.tile([P, P], bf16, tag="ident")
        make_identity(nc, ident[:])
        ones128 = constp.tile([P, 1], bf16, tag="ones128")
        nc.gpsimd.memset(ones128[:], 1.0)
        onesk1 = constp.tile([1, P], bf16, tag="onesk1")
        nc.gpsimd.memset(onesk1[:], 1.0)

        qkvT = [pp.tile([P, S], bf16, tag=f"qkv{i}", name=f"qkv{i}")
                for i in range(5)]
        v_sb = pp.tile([P, 16, P], bf16, tag="v")      # V[k,d] in 16 k-chunks
        ot_sb = pp.tile([P, QH, S], bf16, tag="ot")    # O^T per head [d, s]

        _phase1(tc, d, ident, qkvT, v_sb)

        with tc.tile_pool(name="wo", bufs=1) as wop:
            woutt = [wop.tile([P, HIDDEN], bf16, tag=f"wo{c}", name=f"wo{c}")
                     for c in range(QH)]
            for c in range(QH):
                nc.sync.dma_start(out=woutt[c][:],
                                  in_=d["wout"][c * P:(c + 1) * P, :])
            _phase2(tc, d, ones128, onesk1, qkvT, v_sb, ot_sb)
            _phase3(tc, d, woutt, ot_sb)


def _legalize_waits(nc):
    """This toolchain's codegen accepts at most ONE sync wait per
    instruction; hoist extra waits onto single-wait Drain clones inserted
    just before the instruction on the same engine."""
    import copy
    f = nc.m.functions[0]
    templates = {}
    for blk in f.blocks:
        for inst in blk.instructions:
            if type(inst).__name__ == "InstDrain":
                templates.setdefault(str(inst.engine), inst)
    anyt = next(iter(templates.values()))
    SI = type(next(i for b in f.blocks for i in b.instructions
                   if i.sync_info).sync_info)
    k = 0
    for blk in f.blocks:
        newl = []
        for inst in blk.instructions:
            si = inst.sync_info
            if si and len(si.on_wait) > 1:
                for w in si.on_wait:
                    dcl = copy.deepcopy(templates.get(str(inst.engine), anyt))
                    dcl.engine = inst.engine
                    dcl.name = f"{inst.name}w{k}"; k += 1
                    dcl.sync_info = SI(on_wait=[w], on_update=[])
                    newl.append(dcl)
                inst.sync_info = SI(on_wait=[], on_update=list(si.on_update))
            newl.append(inst)
        try:
            blk.instructions[:] = newl
        except Exception:
            blk.instructions = newl


def build():
    nc = bass.Bass()
    d = {
        "xq": nc.dram_tensor("xq", [NQ, P, HCH, SQ], bf16, kind="ExternalInput"),
        "wq": nc.dram_tensor("wq", [AT, P, HCH, P], bf16, kind="ExternalInput"),
        "wout": nc.dram_tensor("wout", [QH * P, HIDDEN], bf16,
                               kind="ExternalInput"),
        "ctab": nc.dram_tensor("ctab", [P, S], f32, kind="ExternalInput"),
        "stab": nc.dram_tensor("stab", [ROT, S], f32, kind="ExternalInput"),
        "perm": nc.dram_tensor("perm", [ROT, ROT], f32, kind="ExternalInput"),
        "y": nc.dram_tensor("y", [S, HIDDEN], bf16, kind="ExternalOutput"),
    }
    with tile.TileContext(nc) as tc:
        _emit(tc, d)
    _legalize_waits(nc)
    return nc


_NC_CACHE = {}


def _get_nc():
    if "nc" not in _NC_CACHE:
        _NC_CACHE["nc"] = build()
    return _NC_CACHE["nc"]


def make_in_maps(x, W_qkv, W_out):
    xT = np.ascontiguousarray(x.reshape(S, HIDDEN).T)
    # [4096, 2048] -> (c p) (q s) -> [q, p, c, s]
    xq = np.ascontiguousarray(
        xT.reshape(HCH, P, NQ, SQ).transpose(2, 1, 0, 3)).astype(BF16)

    perm = np.concatenate([np.arange(0, ROT, 2), np.arange(1, ROT, 2),
                           np.arange(ROT, P)])
    scale = np.float32(P ** -0.5)

    inv = (10000.0 ** (-np.arange(32) / 32.0)).astype(np.float64)
    fr = np.arange(S, dtype=np.float64)[:, None] * inv[None, :]
    cos = np.cos(fr).T.astype(np.float32)   # [32, S]
    sin = np.sin(fr).T.astype(np.float32)
    ctab = np.concatenate([cos, cos, np.ones((ROT, S), np.float32)], axis=0)
    stab = np.concatenate([-sin, sin], axis=0)
    # swap matrix: swp[m, :] = stage[(m+32) % 64, :]
    permm = np.zeros((ROT, ROT), np.float32)
    for m in range(ROT):
        permm[(m + 32) % ROT, m] = 1.0

    in_maps = []
    for g in range(N_CORES):
        kcol = W_qkv[:, (32 + g) * P:(33 + g) * P][:, perm]
        vcol = W_qkv[:, (40 + g) * P:(41 + g) * P]
        qcols = [W_qkv[:, (4 * g + i) * P:(4 * g + i + 1) * P][:, perm] * scale
                 for i in range(QH)]
        wc = np.stack([kcol, vcol] + qcols, axis=0)          # [6, 4096, 128]
        wq = np.ascontiguousarray(
            wc.reshape(AT, HCH, P, P).transpose(0, 2, 1, 3)).astype(BF16)
        wout = np.ascontiguousarray(
            W_out[g * QH * P:(g + 1) * QH * P, :]).astype(BF16)
        in_maps.append({"xq": xq, "wq": wq, "wout": wout,
                        "ctab": ctab, "stab": stab, "perm": permm})
    return in_maps


def run(x, W_qkv, W_out, trace=False):
    nc = _get_nc()
    in_maps = make_in_maps(np.asarray(x, dtype=np.float32),
                           np.asarray(W_qkv, dtype=np.float32),
                           np.asarray(W_out, dtype=np.float32))
    res = run_bass_kernel_spmd(nc, in_maps, list(range(N_CORES)), trace=trace)
    y = np.zeros((S, HIDDEN), np.float32)
    for om in res.results:
        y += np.asarray(om["y"], dtype=np.float32)
    return y.reshape(1, S, HIDDEN), res


def kernel(x, W_qkv, W_out):
    y, _ = run(x, W_qkv, W_out)
    return y


# revision 24
# speedup vs baseline: 1.1479x; 1.0844x over previous
"""GQA attention block on 8 Trainium2 NeuronCores (tensor-parallel by heads).

Shapes (hardcoded): x [1, 2048, 4096], W_qkv [4096, 6144] (32 Q + 8 K + 8 V
heads, head_dim 128), W_out [4096, 4096]. Partial interleaved RoPE over the
first 64 head dims, full (non-causal) softmax.

Sharding: core g owns KV head g and Q heads 4g..4g+3 (W_qkv columns
[K, V, Q0..Q3] = 768, W_out rows 512g..512(g+1)). Each core computes a
partial output projection in bf16; the host sums the 8 partials in f32.

Host-side preprocessing:
  - x passed pre-transposed as x^T [4096, 2048] in bf16, tiled [128, 32, 512].
  - RoPE pairs de-interleaved by permuting W_qkv columns per rot head
    (Q and K permuted identically => scores unchanged); 1/sqrt(128) folded
    into W_q (RoPE is a rotation, commutes with scaling).
  - cos/sin tables precomputed: ctab [128, S] = [cos; cos; ones],
    stab [64, S] = [-sin; sin].

Device pipeline per core (matmuls bf16, f32 PSUM accumulate):
  Phase 1: QKV^T = W^T x^T streamed over 4 s-slices; RoPE finish of the
    previous (slice, head-tile) group is deferred behind the next group's
    matmul stream so the PE never waits on the scalar-engine PSUM copy.
  Phase 2 (per (head, q-slice) unit, software-pipelined over 3 iterations):
    scores^T matmuls write bf16 PSUM (half bank per 128-k chunk); exp runs
    as 4 FD=2048 activations; the softmax denominator is a DVE pairwise
    tree over the 16 exp chunks + a GPSIMD partition_all_reduce (so no
    ones-matmul, no [1,512] reciprocal, no broadcast matmul); A@V
    accumulates f32; normalization is a single DVE divide fused into the
    PSUM->SBUF evacuation of O^T.
  Phase 3: output projection with bf16 SBUF/DMA.
"""

import numpy as np
import ml_dtypes

import concourse.bass as bass
import concourse.bass_isa as bass_isa
import concourse.mybir as mybir
import concourse.tile as tile
from concourse import library_config
from concourse.bass_utils import run_bass_kernel_spmd
from concourse.masks import make_identity

BF16 = ml_dtypes.bfloat16

P = 128
S = 2048
HIDDEN = 4096
HCH = HIDDEN // P          # 32 contraction chunks
SQ = 512                   # s-slice width
NQ = S // SQ               # 4 s-slices
AT = 6                     # a-tiles per core: 0=K, 1=V, 2..5=Q0..Q3
QH = 4                     # q heads per core
ROT = 64
N_CORES = 8
NU = QH * NQ               # 16 attention units

f32 = mybir.dt.float32
bf16 = mybir.dt.bfloat16
MULT = mybir.AluOpType.mult
ADD = mybir.AluOpType.add
EXP = mybir.ActivationFunctionType.Exp


def _phase1(tc, d, ident, qkvT, v_sb):
    nc = tc.nc
    with (
        tc.tile_pool(name="p1const", bufs=1) as cp,
        tc.tile_pool(name="xq", bufs=2) as xqp,
        tc.tile_pool(name="wq", bufs=1) as wqp,
        tc.tile_pool(name="work1", bufs=2) as workp,
        tc.tile_pool(name="ps1", bufs=1, space="PSUM") as psp,
    ):
        # DMA order: first-needed first so the PE can start ASAP.
        wqt = [wqp.tile([P, HCH, P], bf16, tag=f"w{a}", name=f"w{a}")
               for a in range(AT)]
        ctab = cp.tile([P, S], f32, tag="ctab")
        stab = cp.tile([ROT, S], f32, tag="stab")
        perm = cp.tile([ROT, ROT], f32, tag="perm")

        def load_xq_part(t, q, b, parts):
            step = HCH // parts
            nc.sync.dma_start(out=t[:, b * step:(b + 1) * step, :],
                              in_=d["xq"][q, :, b * step:(b + 1) * step, :])

        def load_xq(q, parts=2):
            t = xqp.tile([P, HCH, SQ], bf16, tag="xq", name=f"xq{q}")
            for b in range(parts):
                load_xq_part(t, q, b, parts)
            return t

        # Interleave the startup DMAs in consumption order: the a-th matmul
        # group needs wq[a] and the c-th chunk of xq[0]; constants gate only
        # the first RoPE finish (one full matmul group later).
        xcur = xqp.tile([P, HCH, SQ], bf16, tag="xq", name="xq0")
        nc.sync.dma_start(out=wqt[0][:], in_=d["wq"][0])
        load_xq_part(xcur, 0, 0, 4)
        nc.sync.dma_start(out=wqt[1][:], in_=d["wq"][1])
        nc.sync.dma_start(out=ctab[:], in_=d["ctab"][:])
        load_xq_part(xcur, 0, 1, 4)
        nc.sync.dma_start(out=stab[:], in_=d["stab"][:])
        nc.sync.dma_start(out=perm[:], in_=d["perm"][:])
        nc.sync.dma_start(out=wqt[2][:], in_=d["wq"][2])
        load_xq_part(xcur, 0, 2, 4)
        nc.sync.dma_start(out=wqt[3][:], in_=d["wq"][3])
        load_xq_part(xcur, 0, 3, 4)
        for a in range(4, AT):
            nc.sync.dma_start(out=wqt[a][:], in_=d["wq"][a])

        def finish(q, a, ps):
            sl = slice(q * SQ, (q + 1) * SQ)
            if a == 1:
                # V: cast to bf16, PE-transpose to V[k,d] chunks
                vst = workp.tile([P, SQ], bf16, tag="vst", bufs=2)
                nc.scalar.copy(vst[:], ps[:])
                pst = psp.tile([P, 4, P], bf16, tag="vt", bufs=2)
                for i in range(4):
                    nc.tensor.transpose(pst[:, i, :], vst[:, i * P:(i + 1) * P],
                                        ident[:])
                nc.vector.tensor_copy(v_sb[:, 4 * q:4 * q + 4, :], pst[:])
            else:
                qi = 0 if a == 0 else a - 1
                stage = workp.tile([P, SQ], f32, tag="stage", bufs=2)
                nc.scalar.copy(stage[:], ps[:])
                swp = psp.tile([ROT, SQ], f32, tag="rope", bufs=2)
                nc.tensor.matmul(swp[:], perm[:], stage[0:ROT, :],
                                 start=True, stop=True)
                tmp = workp.tile([ROT, SQ], f32, tag="tmp", bufs=2)
                nc.vector.tensor_tensor(tmp[:], swp[:], stab[:, sl], op=MULT)
                rot = workp.tile([P, SQ], f32, tag="rot", bufs=2)
                nc.vector.tensor_tensor(rot[:], stage[:], ctab[:, sl], op=MULT)
                nc.vector.tensor_tensor(rot[0:ROT, :], rot[0:ROT, :],
                                        tmp[:], op=ADD)
                nc.vector.tensor_copy(qkvT[qi][:, sl], rot[:])

        prev = None
        for q in range(NQ):
            if q + 1 < NQ:
                xnext = load_xq(q + 1)
            for a in range(AT):
                ps = psp.tile([P, SQ], f32, tag="acc", bufs=2)
                for c in range(HCH):
                    nc.tensor.matmul(ps[:], wqt[a][:, c, :], xcur[:, c, :],
                                     start=(c == 0), stop=(c == HCH - 1))
                if prev is not None:
                    finish(*prev)
                prev = (q, a, ps)
            if q + 1 < NQ:
                xcur = xnext
        finish(*prev)


def _phase2(tc, d, ones128, onesk1, qkvT, v_sb, ot_sb):
    """Per unit (head, q-slice), 3-deep software pipeline:
      iter k   : ST+exp of unit k; DVE tree + A@V + colsum-matmul of k-1;
                 recip/bcast/lb-copy/normalize of k-2.
    The single-partition reciprocal (~3.3us DVE) never blocks the PE: the
    bcast matmul that consumes it runs mid-NEXT-iteration."""
    nc = tc.nc
    with (
        tc.tile_pool(name="pt", bufs=1) as ptp,
        tc.tile_pool(name="sum2", bufs=1) as sump,
        tc.tile_pool(name="ps2", bufs=1, space="PSUM") as psp,
    ):
        kT = qkvT[0]
        units = [(h, j) for j in range(NQ) for h in range(QH)]
        pts, ops, lbs, accs, rrs, lps = {}, {}, {}, {}, {}, {}

        def st_group(k, g):
            # 2 k-chunks per group: [128, 2, 512] f32 = 2 PSUM banks,
            # exp drains both with one FD=1024 activation.
            h, j = units[k]
            if g == 0:
                pts[k] = ptp.tile([P, 16, SQ], bf16, tag="pt", bufs=2,
                                  name=f"pt{k}")
            stg = psp.tile([P, 2, SQ], f32, tag="stg", bufs=3, name="stg")
            for i in range(2):
                c = 2 * g + i
                nc.tensor.matmul(stg[:, i, :], kT[:, c * P:(c + 1) * P],
                                 qkvT[1 + h][:, j * SQ:(j + 1) * SQ],
                                 start=True, stop=True)
            nc.scalar.activation(pts[k][:, 2 * g:2 * g + 2, :], stg[:], EXP)

        def tree(k):
            pt = pts[k]
            acc8 = sump.tile([P, 8, SQ], bf16, tag="acc8", bufs=1)
            nc.vector.tensor_tensor(acc8[:], pt[:, 0:8, :], pt[:, 8:16, :],
                                    op=ADD)
            acc4 = sump.tile([P, 4, SQ], bf16, tag="acc4", bufs=1)
            nc.vector.tensor_tensor(acc4[:], acc8[:, 0:4, :], acc8[:, 4:8, :],
                                    op=ADD)
            acc2 = sump.tile([P, 2, SQ], bf16, tag="acc2", bufs=1)
            nc.vector.tensor_tensor(acc2[:], acc4[:, 0:2, :], acc4[:, 2:4, :],
                                    op=ADD)
            accs[k] = sump.tile([P, SQ], bf16, tag="accS", bufs=2,
                                name=f"accS{k}")
            nc.vector.tensor_tensor(accs[k][:], acc2[:, 0, :], acc2[:, 1, :],
                                    op=ADD)

        def colsum(k):
            # single ones-matmul over the tree result -> l in lp[0:1, :]
            lps[k] = psp.tile([P, SQ], f32, tag="lp", bufs=1, name="lp")
            nc.tensor.matmul(lps[k][0:1, :], ones128[:], accs[k][:],
                             start=True, stop=True)

        def recip(k):
            # bf16 output so the broadcast matmul is single-pass bf16
            rrs[k] = sump.tile([1, SQ], bf16, tag="rr", bufs=2, name=f"rr{k}")
            with nc.allow_low_precision(reason="1/l in bf16: 0.4% rel, "
                                        "within the softmax error budget"):
                nc.vector.reciprocal(rrs[k][:], lps[k][0:1, :])

        def bcast(k):
            # broadcast 1/l to 128 partitions via K=1 matmul, then to SBUF
            nc.tensor.matmul(lps[k][:], onesk1[:], rrs[k][:],
                             start=True, stop=True)

        def lcopy(k):
            lbs[k] = sump.tile([P, SQ], f32, tag="lb", bufs=2, name=f"lb{k}")
            nc.scalar.copy(lbs[k][:], lps[k][:])

        def av_chunks(k, c0, c1):
            if c0 == 0:
                ops[k] = psp.tile([P, SQ], f32, tag="op", bufs=1, name=f"op{k}")
            for c in range(c0, c1):
                nc.tensor.matmul(ops[k][:], v_sb[:, c, :], pts[k][:, c, :],
                                 start=(c == 0), stop=(c == 15))

        def norm(k):
            h, j = units[k]
            nc.vector.tensor_tensor(ot_sb[:, h, j * SQ:(j + 1) * SQ],
                                    ops[k][:], lbs[k][:], op=MULT)
            del pts[k], ops[k], lbs[k], accs[k], rrs[k], lps[k]

        # Steady-state iteration k (unit u0=k doing ST/exp, u1=k-1 the
        # sums/A@V, u2=k-2 the normalize):
        #   DVE: norm(u2) first (frees the single op bank for AV(u1)),
        #        tree(u1), recip(u1) (after the mid-iter colsum matmul)
        #   PE : ST groups with AV chunks between them, colsum(u1) after
        #        g4 (tree done), bcast(u1) last (recip just finished)
        #   ACT: the 8 exps, then lcopy(u1) (after bcast)
        for k in range(NU + 3):
            u0, u1, u2 = k, k - 1, k - 2
            live0 = u0 < NU
            live1 = 0 <= u1 < NU
            live2 = 0 <= u2 < NU
            if live2:
                norm(u2)
            if live0:
                st_group(u0, 0)
                st_group(u0, 1)
            if live1:
                tree(u1)
            for g in range(2, 8):
                if live1:
                    av_chunks(u1, 2 * (g - 2), 2 * (g - 1))
                if live0:
                    st_group(u0, g)
                if g == 4 and live1:
                    colsum(u1)
                    recip(u1)
            if live1:
                av_chunks(u1, 12, 16)
                bcast(u1)
                lcopy(u1)


def _phase3(tc, d, woutt, ot_sb):
    nc = tc.nc
    with (
        tc.tile_pool(name="y", bufs=1) as yp,
        tc.tile_pool(name="ps3", bufs=1, space="PSUM") as psp,
    ):
        for i in range(S // P):
            ysb = yp.tile([P, HIDDEN], bf16, tag="ysb", bufs=2)
            for n in range(HIDDEN // SQ):
                yps = psp.tile([P, SQ], f32, tag="yps", bufs=4)
                for hc in range(QH):
                    nc.tensor.matmul(yps[:], ot_sb[:, hc, i * P:(i + 1) * P],
                                     woutt[hc][:, n * SQ:(n + 1) * SQ],
                                     start=(hc == 0), stop=(hc == QH - 1))
                nc.vector.tensor_copy(ysb[:, n * SQ:(n + 1) * SQ], yps[:])
            nc.sync.dma_start(out=d["y"][i * P:(i + 1) * P, :], in_=ysb[:])


def _emit(tc, d):
    nc = tc.nc
    with (
        tc.tile_pool(name="const", bufs=1) as constp,
        tc.tile_pool(name="persist", bufs=1) as pp,
    ):
        ident = constp.tile([P, P], bf16, tag="ident")
        make_identity(nc, ident[:])
        ones128 = constp.tile([P, 1], bf16, tag="ones128")
        nc.gpsimd.memset(ones128[:], 1.0)
        onesk1 = constp.tile([1, P], bf16, tag="onesk1")
        nc.gpsimd.memset(onesk1[:], 1.0)

        qkvT = [pp.tile([P, S], bf16, tag=f"qkv{i}", name=f"qkv{i}")
                for i in range(5)]
        v_sb = pp.tile([P, 16, P], bf16, tag="v")      # V[k,d] in 16 k-chunks
        ot_sb = pp.tile([P, QH, S], bf16, tag="ot")    # O^T per head [d, s]

        _phase1(tc, d, ident, qkvT, v_sb)

        with tc.tile_pool(name="wo", bufs=1) as wop:
            woutt = [wop.tile([P, HIDDEN], bf16, tag=f"wo{c}", name=f"wo{c}")
                     for c in range(QH)]
            for c in range(QH):
                nc.sync.dma_start(out=woutt[c][:],
                                  in_=d["wout"][c * P:(c + 1) * P, :])
            _phase2(tc, d, ones128, onesk1, qkvT, v_sb, ot_sb)
            _phase3(tc, d, woutt, ot_sb)


def _legalize_waits(nc):
    """This toolchain's codegen accepts at most ONE sync wait per
    instruction; hoist extra waits onto single-wait Drain clones inserted
    just before the instruction on the same engine."""
    import copy
    f = nc.m.functions[0]
    templates = {}
    for blk in f.blocks:
        for inst in blk.instructions:
            if type(inst).__name__ == "InstDrain":
                templates.setdefault(str(inst.engine), inst)
    anyt = next(iter(templates.values()))
    SI = type(next(i for b in f.blocks for i in b.instructions
                   if i.sync_info).sync_info)
    k = 0
    for blk in f.blocks:
        newl = []
        for inst in blk.instructions:
            si = inst.sync_info
            if si and len(si.on_wait) > 1:
                for w in si.on_wait:
                    dcl = copy.deepcopy(templates.get(str(inst.engine), anyt))
                    dcl.engine = inst.engine
                    dcl.name = f"{inst.name}w{k}"; k += 1
                    dcl.sync_info = SI(on_wait=[w], on_update=[])
                    newl.append(dcl)
                inst.sync_info = SI(on_wait=[], on_update=list(si.on_update))
            newl.append(inst)
        try:
            blk.instructions[:] = newl
        except Exception:
            blk.instructions = newl


def build():
    nc = bass.Bass()
    d = {
        "xq": nc.dram_tensor("xq", [NQ, P, HCH, SQ], bf16, kind="ExternalInput"),
        "wq": nc.dram_tensor("wq", [AT, P, HCH, P], bf16, kind="ExternalInput"),
        "wout": nc.dram_tensor("wout", [QH * P, HIDDEN], bf16,
                               kind="ExternalInput"),
        "ctab": nc.dram_tensor("ctab", [P, S], f32, kind="ExternalInput"),
        "stab": nc.dram_tensor("stab", [ROT, S], f32, kind="ExternalInput"),
        "perm": nc.dram_tensor("perm", [ROT, ROT], f32, kind="ExternalInput"),
        "y": nc.dram_tensor("y", [S, HIDDEN], bf16, kind="ExternalOutput"),
    }
    with tile.TileContext(nc) as tc:
        _emit(tc, d)
    _legalize_waits(nc)
    return nc


_NC_CACHE = {}


def _get_nc():
    if "nc" not in _NC_CACHE:
        _NC_CACHE["nc"] = build()
    return _NC_CACHE["nc"]


def make_in_maps(x, W_qkv, W_out):
    xT = np.ascontiguousarray(x.reshape(S, HIDDEN).T)
    # [4096, 2048] -> (c p) (q s) -> [q, p, c, s]
    xq = np.ascontiguousarray(
        xT.reshape(HCH, P, NQ, SQ).transpose(2, 1, 0, 3)).astype(BF16)

    perm = np.concatenate([np.arange(0, ROT, 2), np.arange(1, ROT, 2),
                           np.arange(ROT, P)])
    scale = np.float32(P ** -0.5)

    inv = (10000.0 ** (-np.arange(32) / 32.0)).astype(np.float64)
    fr = np.arange(S, dtype=np.float64)[:, None] * inv[None, :]
    cos = np.cos(fr).T.astype(np.float32)   # [32, S]
    sin = np.sin(fr).T.astype(np.float32)
    ctab = np.concatenate([cos, cos, np.ones((ROT, S), np.float32)], axis=0)
    stab = np.concatenate([-sin, sin], axis=0)
    # swap matrix: swp[m, :] = stage[(m+32) % 64, :]
    permm = np.zeros((ROT, ROT), np.float32)
    for m in range(ROT):
        permm[(m + 32) % ROT, m] = 1.0

    in_maps = []
    for g in range(N_CORES):
        kcol = W_qkv[:, (32 + g) * P:(33 + g) * P][:, perm]
        vcol = W_qkv[:, (40 + g) * P:(41 + g) * P]
        qcols = [W_qkv[:, (4 * g + i) * P:(4 * g + i + 1) * P][:, perm] * scale
                 for i in range(QH)]
        wc = np.stack([kcol, vcol] + qcols, axis=0)          # [6, 4096, 128]
        wq = np.ascontiguousarray(
            wc.reshape(AT, HCH, P, P).transpose(0, 2, 1, 3)).astype(BF16)
        wout = np.ascontiguousarray(
            W_out[g * QH * P:(g + 1) * QH * P, :]).astype(BF16)
        in_maps.append({"xq": xq, "wq": wq, "wout": wout,
                        "ctab": ctab, "stab": stab, "perm": permm})
    return in_maps


def run(x, W_qkv, W_out, trace=False):
    nc = _get_nc()
    in_maps = make_in_maps(np.asarray(x, dtype=np.float32),
                           np.asarray(W_qkv, dtype=np.float32),
                           np.asarray(W_out, dtype=np.float32))
    res = run_bass_kernel_spmd(nc, in_maps, list(range(N_CORES)), trace=trace)
    y = np.zeros((S, HIDDEN), np.float32)
    for om in res.results:
        y += np.asarray(om["y"], dtype=np.float32)
    return y.reshape(1, S, HIDDEN), res


def kernel(x, W_qkv, W_out):
    y, _ = run(x, W_qkv, W_out)
    return y


# revision 29
# speedup vs baseline: 1.1563x; 1.0073x over previous
"""GQA attention block on 8 Trainium2 NeuronCores (tensor-parallel by heads).

Shapes (hardcoded): x [1, 2048, 4096], W_qkv [4096, 6144] (32 Q + 8 K + 8 V
heads, head_dim 128), W_out [4096, 4096]. Partial interleaved RoPE over the
first 64 head dims, full (non-causal) softmax.

Sharding: core g owns KV head g and Q heads 4g..4g+3 (W_qkv columns
[K, V, Q0..Q3] = 768, W_out rows 512g..512(g+1)). Each core computes a
partial output projection in bf16; the host sums the 8 partials in f32.

Host-side preprocessing:
  - x passed pre-transposed as x^T [4096, 2048] in bf16, tiled [128, 32, 512].
  - RoPE pairs de-interleaved by permuting W_qkv columns per rot head
    (Q and K permuted identically => scores unchanged); 1/sqrt(128) folded
    into W_q (RoPE is a rotation, commutes with scaling).
  - cos/sin tables precomputed: ctab [128, S] = [cos; cos; ones],
    stab [64, S] = [-sin; sin].

Device pipeline per core (matmuls bf16, f32 PSUM accumulate):
  Phase 1: QKV^T = W^T x^T streamed over 4 s-slices; RoPE finish of the
    previous (slice, head-tile) group is deferred behind the next group's
    matmul stream so the PE never waits on the scalar-engine PSUM copy.
  Phase 2 (per (head, q-slice) unit, software-pipelined over 3 iterations):
    scores^T matmuls write bf16 PSUM (half bank per 128-k chunk); exp runs
    as 4 FD=2048 activations; the softmax denominator is a DVE pairwise
    tree over the 16 exp chunks + a GPSIMD partition_all_reduce (so no
    ones-matmul, no [1,512] reciprocal, no broadcast matmul); A@V
    accumulates f32; normalization is a single DVE divide fused into the
    PSUM->SBUF evacuation of O^T.
  Phase 3: output projection with bf16 SBUF/DMA.
"""

import numpy as np
import ml_dtypes

import concourse.bass as bass
import concourse.bass_isa as bass_isa
import concourse.mybir as mybir
import concourse.tile as tile
from concourse import library_config
from concourse.bass_utils import run_bass_kernel_spmd
from concourse.masks import make_identity

BF16 = ml_dtypes.bfloat16

P = 128
S = 2048
HIDDEN = 4096
HCH = HIDDEN // P          # 32 contraction chunks
SQ = 512                   # s-slice width
NQ = S // SQ               # 4 s-slices
AT = 6                     # a-tiles per core: 0=K, 1=V, 2..5=Q0..Q3
QH = 4                     # q heads per core
ROT = 64
N_CORES = 8
NU = QH * NQ               # 16 attention units

f32 = mybir.dt.float32
bf16 = mybir.dt.bfloat16
MULT = mybir.AluOpType.mult
ADD = mybir.AluOpType.add
EXP = mybir.ActivationFunctionType.Exp


def _phase1(tc, d, ident, qkvT, v_sb):
    nc = tc.nc
    with (
        tc.tile_pool(name="p1const", bufs=1) as cp,
        tc.tile_pool(name="xq", bufs=2) as xqp,
        tc.tile_pool(name="wq", bufs=1) as wqp,
        tc.tile_pool(name="work1", bufs=2) as workp,
        tc.tile_pool(name="ps1", bufs=1, space="PSUM") as psp,
    ):
        # DMA order: first-needed first so the PE can start ASAP.
        wqt = [wqp.tile([P, HCH, P], bf16, tag=f"w{a}", name=f"w{a}")
               for a in range(AT)]
        ctab = cp.tile([P, S], f32, tag="ctab")
        stab = cp.tile([ROT, S], f32, tag="stab")
        perm = cp.tile([ROT, ROT], f32, tag="perm")

        def load_xq_part(t, q, b, parts):
            step = HCH // parts
            nc.sync.dma_start(out=t[:, b * step:(b + 1) * step, :],
                              in_=d["xq"][q, :, b * step:(b + 1) * step, :])

        def load_xq(q, parts=2):
            t = xqp.tile([P, HCH, SQ], bf16, tag="xq", name=f"xq{q}")
            for b in range(parts):
                load_xq_part(t, q, b, parts)
            return t

        # Interleave the startup DMAs in consumption order: the a-th matmul
        # group needs wq[a] and the c-th chunk of xq[0]; constants gate only
        # the first RoPE finish (one full matmul group later).
        xcur = xqp.tile([P, HCH, SQ], bf16, tag="xq", name="xq0")
        nc.sync.dma_start(out=wqt[0][:], in_=d["wq"][0])
        load_xq_part(xcur, 0, 0, 4)
        nc.sync.dma_start(out=wqt[1][:], in_=d["wq"][1])
        nc.sync.dma_start(out=ctab[:], in_=d["ctab"][:])
        load_xq_part(xcur, 0, 1, 4)
        nc.sync.dma_start(out=stab[:], in_=d["stab"][:])
        nc.sync.dma_start(out=perm[:], in_=d["perm"][:])
        nc.sync.dma_start(out=wqt[2][:], in_=d["wq"][2])
        load_xq_part(xcur, 0, 2, 4)
        nc.sync.dma_start(out=wqt[3][:], in_=d["wq"][3])
        load_xq_part(xcur, 0, 3, 4)
        for a in range(4, AT):
            nc.sync.dma_start(out=wqt[a][:], in_=d["wq"][a])

        def finish(q, a, ps):
            sl = slice(q * SQ, (q + 1) * SQ)
            if a == 1:
                # V: cast to bf16, PE-transpose to V[k,d] chunks
                vst = workp.tile([P, SQ], bf16, tag="vst", bufs=2)
                nc.scalar.copy(vst[:], ps[:])
                pst = psp.tile([P, 4, P], bf16, tag="vt", bufs=2)
                for i in range(4):
                    nc.tensor.transpose(pst[:, i, :], vst[:, i * P:(i + 1) * P],
                                        ident[:])
                nc.vector.tensor_copy(v_sb[:, 4 * q:4 * q + 4, :], pst[:])
            else:
                qi = 0 if a == 0 else a - 1
                stage = workp.tile([P, SQ], f32, tag="stage", bufs=2)
                nc.scalar.copy(stage[:], ps[:])
                swp = psp.tile([ROT, SQ], f32, tag="rope", bufs=2)
                nc.tensor.matmul(swp[:], perm[:], stage[0:ROT, :],
                                 start=True, stop=True)
                tmp = workp.tile([ROT, SQ], f32, tag="tmp", bufs=2)
                nc.vector.tensor_tensor(tmp[:], swp[:], stab[:, sl], op=MULT)
                rot = workp.tile([P, SQ], f32, tag="rot", bufs=2)
                nc.vector.tensor_tensor(rot[:], stage[:], ctab[:, sl], op=MULT)
                nc.vector.tensor_tensor(rot[0:ROT, :], rot[0:ROT, :],
                                        tmp[:], op=ADD)
                nc.vector.tensor_copy(qkvT[qi][:, sl], rot[:])

        prev = None
        for q in range(NQ):
            if q + 1 < NQ:
                xnext = load_xq(q + 1)
            for a in range(AT):
                ps = psp.tile([P, SQ], f32, tag="acc", bufs=2)
                for c in range(HCH):
                    nc.tensor.matmul(ps[:], wqt[a][:, c, :], xcur[:, c, :],
                                     start=(c == 0), stop=(c == HCH - 1))
                if prev is not None:
                    finish(*prev)
                prev = (q, a, ps)
            if q + 1 < NQ:
                xcur = xnext
        finish(*prev)


def _phase2(tc, d, ones128, onesk1, qkvT, v_sb, ot_sb, woutt):
    """Per unit (head, q-slice), 3-deep software pipeline:
      iter k   : ST+exp of unit k; DVE tree + A@V + colsum-matmul of k-1;
                 recip/bcast/lb-copy/normalize of k-2.
    The single-partition reciprocal (~3.3us DVE) never blocks the PE: the
    bcast matmul that consumes it runs mid-NEXT-iteration."""
    nc = tc.nc
    with (
        tc.tile_pool(name="pt", bufs=1) as ptp,
        tc.tile_pool(name="sum2", bufs=1) as sump,
        tc.tile_pool(name="y", bufs=1) as yp,
        tc.tile_pool(name="ps2", bufs=1, space="PSUM") as psp,
    ):
        kT = qkvT[0]
        units = [(h, j) for j in range(NQ) for h in range(QH)]
        pts, ops, lbs, accs, rrs, lps = {}, {}, {}, {}, {}, {}

        # --- output-projection sub-blocks, interleaved into PE slack ---
        # sub-block (i, n): y[i*128:(i+1)*128, n*512:(n+1)*512]; row-block i
        # needs units 4*(i//4)..4*(i//4)+3 normalized (issued by iter
        # 4*(i//4)+5). yps shares the "stg" PSUM ring.
        p3q = [(i, n) for i in range(S // P) for n in range(HIDDEN // SQ)]
        p3 = [0]
        ysbs = {}

        def ph3_sub(k):
            if p3[0] >= len(p3q):
                return
            i, n = p3q[p3[0]]
            if k < 4 * (i // 4) + 6:
                return
            if n == 0:
                ysbs[i] = yp.tile([P, HIDDEN], bf16, tag="ysb", bufs=2,
                                  name=f"ysb{i}")
            yps = psp.tile([P, 2, SQ], f32, tag="stg", bufs=3, name="yps")
            for hc in range(QH):
                nc.tensor.matmul(yps[:, 0, :], ot_sb[:, hc, i * P:(i + 1) * P],
                                 woutt[hc][:, n * SQ:(n + 1) * SQ],
                                 start=(hc == 0), stop=(hc == QH - 1))
            nc.vector.tensor_copy(ysbs[i][:, n * SQ:(n + 1) * SQ], yps[:, 0, :])
            if n == HIDDEN // SQ - 1:
                nc.sync.dma_start(out=d["y"][i * P:(i + 1) * P, :],
                                  in_=ysbs[i][:])
            p3[0] += 1

        def st_group(k, g):
            # 2 k-chunks per group: [128, 2, 512] f32 = 2 PSUM banks,
            # exp drains both with one FD=1024 activation.
            h, j = units[k]
            if g == 0:
                pts[k] = ptp.tile([P, 16, SQ], bf16, tag="pt", bufs=2,
                                  name=f"pt{k}")
            stg = psp.tile([P, 2, SQ], f32, tag="stg", bufs=3, name="stg")
            for i in range(2):
                c = 2 * g + i
                nc.tensor.matmul(stg[:, i, :], kT[:, c * P:(c + 1) * P],
                                 qkvT[1 + h][:, j * SQ:(j + 1) * SQ],
                                 start=True, stop=True)
            nc.scalar.activation(pts[k][:, 2 * g:2 * g + 2, :], stg[:], EXP)

        def tree(k):
            pt = pts[k]
            acc8 = sump.tile([P, 8, SQ], bf16, tag="acc8", bufs=1)
            nc.vector.tensor_tensor(acc8[:], pt[:, 0:8, :], pt[:, 8:16, :],
                                    op=ADD)
            acc4 = sump.tile([P, 4, SQ], bf16, tag="acc4", bufs=1)
            nc.vector.tensor_tensor(acc4[:], acc8[:, 0:4, :], acc8[:, 4:8, :],
                                    op=ADD)
            acc2 = sump.tile([P, 2, SQ], bf16, tag="acc2", bufs=1)
            nc.vector.tensor_tensor(acc2[:], acc4[:, 0:2, :], acc4[:, 2:4, :],
                                    op=ADD)
            accs[k] = sump.tile([P, SQ], bf16, tag="accS", bufs=2,
                                name=f"accS{k}")
            nc.vector.tensor_tensor(accs[k][:], acc2[:, 0, :], acc2[:, 1, :],
                                    op=ADD)

        def colsum(k):
            # single ones-matmul over the tree result -> l in lp[0:1, :]
            lps[k] = psp.tile([P, SQ], f32, tag="lp", bufs=1, name="lp")
            nc.tensor.matmul(lps[k][0:1, :], ones128[:], accs[k][:],
                             start=True, stop=True)

        def recip(k):
            # bf16 output so the broadcast matmul is single-pass bf16
            rrs[k] = sump.tile([1, SQ], bf16, tag="rr", bufs=2, name=f"rr{k}")
            with nc.allow_low_precision(reason="1/l in bf16: 0.4% rel, "
                                        "within the softmax error budget"):
                nc.vector.reciprocal(rrs[k][:], lps[k][0:1, :])

        def bcast(k):
            # broadcast 1/l to 128 partitions via K=1 matmul, then to SBUF
            nc.tensor.matmul(lps[k][:], onesk1[:], rrs[k][:],
                             start=True, stop=True)

        def lcopy(k):
            lbs[k] = sump.tile([P, SQ], f32, tag="lb", bufs=2, name=f"lb{k}")
            nc.scalar.copy(lbs[k][:], lps[k][:])

        def av_chunks(k, c0, c1):
            if c0 == 0:
                ops[k] = psp.tile([P, SQ], f32, tag="op", bufs=1, name=f"op{k}")
            for c in range(c0, c1):
                nc.tensor.matmul(ops[k][:], v_sb[:, c, :], pts[k][:, c, :],
                                 start=(c == 0), stop=(c == 15))

        def norm(k):
            h, j = units[k]
            nc.vector.tensor_tensor(ot_sb[:, h, j * SQ:(j + 1) * SQ],
                                    ops[k][:], lbs[k][:], op=MULT)
            del pts[k], ops[k], lbs[k], accs[k], rrs[k], lps[k]

        # Steady-state iteration k (unit u0=k doing ST/exp, u1=k-1 the
        # sums/A@V, u2=k-2 the normalize):
        #   DVE: norm(u2) first (frees the single op bank for AV(u1)),
        #        tree(u1), recip(u1) (after the mid-iter colsum matmul)
        #   PE : ST groups with AV chunks between them, colsum(u1) after
        #        g4 (tree done), bcast(u1) last (recip just finished)
        #   ACT: the 8 exps, then lcopy(u1) (after bcast)
        for k in range(NU + 3):
            u0, u1, u2 = k, k - 1, k - 2
            live0 = u0 < NU
            live1 = 0 <= u1 < NU
            live2 = 0 <= u2 < NU
            if live2:
                norm(u2)
            if live0:
                st_group(u0, 0)
                st_group(u0, 1)
            if live1:
                tree(u1)
            for g in range(2, 8):
                if live1:
                    av_chunks(u1, 2 * (g - 2), 2 * (g - 1))
                if live0:
                    st_group(u0, g)
                if g == 4 and live1:
                    colsum(u1)
                    recip(u1)
                if g == 5:
                    ph3_sub(k)
            if live1:
                av_chunks(u1, 12, 16)
                bcast(u1)
                lcopy(u1)
            ph3_sub(k)

        # remaining output-projection sub-blocks (dense tail)
        while p3[0] < len(p3q):
            ph3_sub(10 ** 9)


def _emit(tc, d):
    nc = tc.nc
    with (
        tc.tile_pool(name="const", bufs=1) as constp,
        tc.tile_pool(name="persist", bufs=1) as pp,
    ):
        ident = constp.tile([P, P], bf16, tag="ident")
        make_identity(nc, ident[:])
        ones128 = constp.tile([P, 1], bf16, tag="ones128")
        nc.gpsimd.memset(ones128[:], 1.0)
        onesk1 = constp.tile([1, P], bf16, tag="onesk1")
        nc.gpsimd.memset(onesk1[:], 1.0)

        qkvT = [pp.tile([P, S], bf16, tag=f"qkv{i}", name=f"qkv{i}")
                for i in range(5)]
        v_sb = pp.tile([P, 16, P], bf16, tag="v")      # V[k,d] in 16 k-chunks
        ot_sb = pp.tile([P, QH, S], bf16, tag="ot")    # O^T per head [d, s]

        _phase1(tc, d, ident, qkvT, v_sb)

        with tc.tile_pool(name="wo", bufs=1) as wop:
            woutt = [wop.tile([P, HIDDEN], bf16, tag=f"wo{c}", name=f"wo{c}")
                     for c in range(QH)]
            for c in range(QH):
                nc.sync.dma_start(out=woutt[c][:],
                                  in_=d["wout"][c * P:(c + 1) * P, :])
            _phase2(tc, d, ones128, onesk1, qkvT, v_sb, ot_sb, woutt)


def _legalize_waits(nc):
    """This toolchain's codegen accepts at most ONE sync wait per
    instruction; hoist extra waits onto single-wait Drain clones inserted
    just before the instruction on the same engine."""
    import copy
    f = nc.m.functions[0]
    templates = {}
    for blk in f.blocks:
        for inst in blk.instructions:
            if type(inst).__name__ == "InstDrain":
                templates.setdefault(str(inst.engine), inst)
    anyt = next(iter(templates.values()))
    SI = type(next(i for b in f.blocks for i in b.instructions
                   if i.sync_info).sync_info)
    k = 0
    for blk in f.blocks:
        newl = []
        for inst in blk.instructions:
            si = inst.sync_info
            if si and len(si.on_wait) > 1:
                for w in si.on_wait:
                    dcl = copy.deepcopy(templates.get(str(inst.engine), anyt))
                    dcl.engine = inst.engine
                    dcl.name = f"{inst.name}w{k}"; k += 1
                    dcl.sync_info = SI(on_wait=[w], on_update=[])
                    newl.append(dcl)
                inst.sync_info = SI(on_wait=[], on_update=list(si.on_update))
            newl.append(inst)
        try:
            blk.instructions[:] = newl
        except Exception:
            blk.instructions = newl


def build():
    nc = bass.Bass()
    d = {
        "xq": nc.dram_tensor("xq", [NQ, P, HCH, SQ], bf16, kind="ExternalInput"),
        "wq": nc.dram_tensor("wq", [AT, P, HCH, P], bf16, kind="ExternalInput"),
        "wout": nc.dram_tensor("wout", [QH * P, HIDDEN], bf16,
                               kind="ExternalInput"),
        "ctab": nc.dram_tensor("ctab", [P, S], f32, kind="ExternalInput"),
        "stab": nc.dram_tensor("stab", [ROT, S], f32, kind="ExternalInput"),
        "perm": nc.dram_tensor("perm", [ROT, ROT], f32, kind="ExternalInput"),
        "y": nc.dram_tensor("y", [S, HIDDEN], bf16, kind="ExternalOutput"),
    }
    with tile.TileContext(nc) as tc:
        _emit(tc, d)
    _legalize_waits(nc)
    return nc


_NC_CACHE = {}


def _get_nc():
    if "nc" not in _NC_CACHE:
        _NC_CACHE["nc"] = build()
    return _NC_CACHE["nc"]


def make_in_maps(x, W_qkv, W_out):
    xT = np.ascontiguousarray(x.reshape(S, HIDDEN).T)
    # [4096, 2048] -> (c p) (q s) -> [q, p, c, s]
    xq = np.ascontiguousarray(
        xT.reshape(HCH, P, NQ, SQ).transpose(2, 1, 0, 3)).astype(BF16)

    perm = np.concatenate([np.arange(0, ROT, 2), np.arange(1, ROT, 2),
                           np.arange(ROT, P)])
    scale = np.float32(P ** -0.5)

    inv = (10000.0 ** (-np.arange(32) / 32.0)).astype(np.float64)
    fr = np.arange(S, dtype=np.float64)[:, None] * inv[None, :]
    cos = np.cos(fr).T.astype(np.float32)   # [32, S]
    sin = np.sin(fr).T.astype(np.float32)
    ctab = np.concatenate([cos, cos, np.ones((ROT, S), np.float32)], axis=0)
    stab = np.concatenate([-sin, sin], axis=0)
    # swap matrix: swp[m, :] = stage[(m+32) % 64, :]
    permm = np.zeros((ROT, ROT), np.float32)
    for m in range(ROT):
        permm[(m + 32) % ROT, m] = 1.0

    in_maps = []
    for g in range(N_CORES):
        kcol = W_qkv[:, (32 + g) * P:(33 + g) * P][:, perm]
        vcol = W_qkv[:, (40 + g) * P:(41 + g) * P]
        qcols = [W_qkv[:, (4 * g + i) * P:(4 * g + i + 1) * P][:, perm] * scale
                 for i in range(QH)]
        wc = np.stack([kcol, vcol] + qcols, axis=0)          # [6, 4096, 128]
        wq = np.ascontiguousarray(
            wc.reshape(AT, HCH, P, P).transpose(0, 2, 1, 3)).astype(BF16)
        wout = np.ascontiguousarray(
            W_out[g * QH * P:(g + 1) * QH * P, :]).astype(BF16)
        in_maps.append({"xq": xq, "wq": wq, "wout": wout,
                        "ctab": ctab, "stab": stab, "perm": permm})
    return in_maps


def run(x, W_qkv, W_out, trace=False):
    nc = _get_nc()
    in_maps = make_in_maps(np.asarray(x, dtype=np.float32),
                           np.asarray(W_qkv, dtype=np.float32),
                           np.asarray(W_out, dtype=np.float32))
    res = run_bass_kernel_spmd(nc, in_maps, list(range(N_CORES)), trace=trace)
    y = np.zeros((S, HIDDEN), np.float32)
    for om in res.results:
        y += np.asarray(om["y"], dtype=np.float32)
    return y.reshape(1, S, HIDDEN), res


def kernel(x, W_qkv, W_out):
    y, _ = run(x, W_qkv, W_out)
    return y


# revision 35
# speedup vs baseline: 1.2108x; 1.0471x over previous
"""GQA attention block on 8 Trainium2 NeuronCores (tensor-parallel by heads).

Shapes (hardcoded): x [1, 2048, 4096], W_qkv [4096, 6144] (32 Q + 8 K + 8 V
heads, head_dim 128), W_out [4096, 4096]. Partial interleaved RoPE over the
first 64 head dims, full (non-causal) softmax.

Sharding: core g owns KV head g and Q heads 4g..4g+3 (W_qkv columns
[K, V, Q0..Q3] = 768, W_out rows 512g..512(g+1)). Each core computes a
partial output projection in bf16; the host sums the 8 partials in f32.

Host-side preprocessing:
  - x passed pre-transposed as x^T [4096, 2048] in bf16, tiled [128, 32, 512].
  - RoPE pairs de-interleaved by permuting W_qkv columns per rot head
    (Q and K permuted identically => scores unchanged); 1/sqrt(128) folded
    into W_q (RoPE is a rotation, commutes with scaling).
  - cos/sin tables precomputed: ctab [128, S] = [cos; cos; ones],
    stab [64, S] = [-sin; sin].

Device pipeline per core (matmuls bf16, f32 PSUM accumulate):
  Phase 1: QKV^T = W^T x^T streamed over 4 s-slices; RoPE finish of the
    previous (slice, head-tile) group is deferred behind the next group's
    matmul stream so the PE never waits on the scalar-engine PSUM copy.
  Phase 2 (per (head, q-slice) unit, software-pipelined over 3 iterations):
    scores^T matmuls write bf16 PSUM (half bank per 128-k chunk); exp runs
    as 4 FD=2048 activations; the softmax denominator is a DVE pairwise
    tree over the 16 exp chunks + a GPSIMD partition_all_reduce (so no
    ones-matmul, no [1,512] reciprocal, no broadcast matmul); A@V
    accumulates f32; normalization is a single DVE divide fused into the
    PSUM->SBUF evacuation of O^T.
  Phase 3: output projection with bf16 SBUF/DMA.
"""

import numpy as np
import ml_dtypes

import concourse.bass as bass
import concourse.bass_isa as bass_isa
import concourse.mybir as mybir
import concourse.tile as tile
from concourse import library_config
from concourse.bass_utils import run_bass_kernel_spmd
from concourse.masks import make_identity

BF16 = ml_dtypes.bfloat16

P = 128
S = 2048
HIDDEN = 4096
HCH = HIDDEN // P          # 32 contraction chunks
SQ = 512                   # s-slice width
NQ = S // SQ               # 4 s-slices
AT = 6                     # a-tiles per core: 0=K, 1=V, 2..5=Q0..Q3
QH = 4                     # q heads per core
ROT = 64
N_CORES = 8
NU = QH * NQ               # 16 attention units

f32 = mybir.dt.float32
bf16 = mybir.dt.bfloat16
MULT = mybir.AluOpType.mult
ADD = mybir.AluOpType.add
EXP = mybir.ActivationFunctionType.Exp


def _phase1(tc, d, ident, qkvT, v_sb):
    nc = tc.nc
    with (
        tc.tile_pool(name="p1const", bufs=1) as cp,
        tc.tile_pool(name="xq", bufs=2) as xqp,
        tc.tile_pool(name="wq", bufs=1) as wqp,
        tc.tile_pool(name="work1", bufs=2) as workp,
        tc.tile_pool(name="ps1", bufs=1, space="PSUM") as psp,
    ):
        # DMA order: first-needed first so the PE can start ASAP.
        wqt = [wqp.tile([P, HCH, P], bf16, tag=f"w{a}", name=f"w{a}")
               for a in range(AT)]
        ctab = cp.tile([P, S], f32, tag="ctab")
        stab = cp.tile([ROT, S], f32, tag="stab")
        perm = cp.tile([ROT, ROT], bf16, tag="perm")

        def load_xq_part(t, q, b, parts):
            step = HCH // parts
            nc.sync.dma_start(out=t[:, b * step:(b + 1) * step, :],
                              in_=d["xq"][q, :, b * step:(b + 1) * step, :])

        def load_xq(q, parts=2):
            t = xqp.tile([P, HCH, SQ], bf16, tag="xq", name=f"xq{q}")
            for b in range(parts):
                load_xq_part(t, q, b, parts)
            return t

        # Startup DMAs in consumption order: the serial DMA stream's critical
        # prefix is wq[0] + the xq[0] chunks (the first matmul group);
        # everything else trails in first-use order.
        xcur = xqp.tile([P, HCH, SQ], bf16, tag="xq", name="xq0")
        nc.sync.dma_start(out=wqt[0][:], in_=d["wq"][0])
        for b in range(4):
            load_xq_part(xcur, 0, b, 4)
        nc.sync.dma_start(out=wqt[1][:], in_=d["wq"][1])
        nc.sync.dma_start(out=ctab[:], in_=d["ctab"][:])
        nc.sync.dma_start(out=stab[:], in_=d["stab"][:])
        nc.sync.dma_start(out=perm[:], in_=d["perm"][:])
        for a in range(2, AT):
            nc.sync.dma_start(out=wqt[a][:], in_=d["wq"][a])

        def finish(q, a, ps):
            sl = slice(q * SQ, (q + 1) * SQ)
            if a == 1:
                # V: cast to bf16, PE-transpose to V[k,d] chunks
                vst = workp.tile([P, SQ], bf16, tag="vst", bufs=2)
                nc.scalar.copy(vst[:], ps[:])
                pst = psp.tile([P, 4, P], bf16, tag="vt", bufs=2)
                for i in range(4):
                    nc.tensor.transpose(pst[:, i, :], vst[:, i * P:(i + 1) * P],
                                        ident[:])
                nc.vector.tensor_copy(v_sb[:, 4 * q:4 * q + 4, :], pst[:])
            else:
                qi = 0 if a == 0 else a - 1
                stage = workp.tile([P, SQ], f32, tag="stage", bufs=2)
                nc.scalar.copy(stage[:], ps[:])
                # bf16 copy of the rot half so the halves-swap matmul is
                # single-pass bf16 (an fp32 matmul costs two passes)
                stage_rot = workp.tile([ROT, SQ], bf16, tag="stage_rot",
                                       bufs=2)
                nc.scalar.copy(stage_rot[:], ps[0:ROT, :])
                swp = psp.tile([ROT, SQ], f32, tag="rope", bufs=2)
                nc.tensor.matmul(swp[:], perm[:], stage_rot[:],
                                 start=True, stop=True)
                tmp = workp.tile([ROT, SQ], f32, tag="tmp", bufs=2)
                nc.vector.tensor_tensor(tmp[:], swp[:], stab[:, sl], op=MULT)
                rot = workp.tile([P, SQ], f32, tag="rot", bufs=2)
                nc.vector.tensor_tensor(rot[:], stage[:], ctab[:, sl], op=MULT)
                nc.vector.tensor_tensor(rot[0:ROT, :], rot[0:ROT, :],
                                        tmp[:], op=ADD)
                nc.vector.tensor_copy(qkvT[qi][:, sl], rot[:])

        prev = None
        for q in range(NQ):
            if q + 1 < NQ:
                xnext = load_xq(q + 1)
            for a in range(AT):
                ps = psp.tile([P, SQ], f32, tag="acc", bufs=2)
                for c in range(HCH):
                    nc.tensor.matmul(ps[:], wqt[a][:, c, :], xcur[:, c, :],
                                     start=(c == 0), stop=(c == HCH - 1))
                if prev is not None:
                    finish(*prev)
                prev = (q, a, ps)
            if q + 1 < NQ:
                xcur = xnext
        finish(*prev)


def _phase2(tc, d, ones128, onesk1, qkvT, v_sb, ot_sb, woutt):
    """Per unit (head, q-slice), 3-deep software pipeline:
      iter k   : ST+exp of unit k; DVE tree + A@V + colsum-matmul of k-1;
                 recip/bcast/lb-copy/normalize of k-2.
    The single-partition reciprocal (~3.3us DVE) never blocks the PE: the
    bcast matmul that consumes it runs mid-NEXT-iteration."""
    nc = tc.nc
    with (
        tc.tile_pool(name="pt", bufs=1) as ptp,
        tc.tile_pool(name="sum2", bufs=1) as sump,
        tc.tile_pool(name="y", bufs=1) as yp,
        tc.tile_pool(name="ps2", bufs=1, space="PSUM") as psp,
    ):
        kT = qkvT[0]
        units = [(h, j) for j in range(NQ) for h in range(QH)]
        pts, ops, lbs, accs, rrs, lps = {}, {}, {}, {}, {}, {}

        # --- output-projection sub-blocks, interleaved into PE slack ---
        # sub-block (i, n): y[i*128:(i+1)*128, n*512:(n+1)*512]; row-block i
        # needs units 4*(i//4)..4*(i//4)+3 normalized (issued by iter
        # 4*(i//4)+5). yps shares the "stg" PSUM ring.
        p3q = [(i, n) for i in range(S // P) for n in range(HIDDEN // SQ)]
        p3 = [0]
        ysbs = {}

        def ph3_sub(k):
            if p3[0] >= len(p3q):
                return
            i, n = p3q[p3[0]]
            if k < 4 * (i // 4) + 6:
                return
            if n == 0:
                ysbs[i] = yp.tile([P, HIDDEN], bf16, tag="ysb", bufs=2,
                                  name=f"ysb{i}")
            yps = psp.tile([P, 2, SQ], f32, tag="stg", bufs=3, name="yps")
            for hc in range(QH):
                nc.tensor.matmul(yps[:, 0, :], ot_sb[:, hc, i * P:(i + 1) * P],
                                 woutt[hc][:, n * SQ:(n + 1) * SQ],
                                 start=(hc == 0), stop=(hc == QH - 1))
            nc.vector.tensor_copy(ysbs[i][:, n * SQ:(n + 1) * SQ], yps[:, 0, :])
            # DMA out in halves so the final transfer is short
            half = HIDDEN // 2
            if n == 3:
                nc.sync.dma_start(out=d["y"][i * P:(i + 1) * P, 0:half],
                                  in_=ysbs[i][:, 0:half])
            elif n == HIDDEN // SQ - 1:
                nc.sync.dma_start(out=d["y"][i * P:(i + 1) * P, half:HIDDEN],
                                  in_=ysbs[i][:, half:HIDDEN])
            p3[0] += 1

        def st_group(k, g):
            # 2 k-chunks per group: [128, 2, 512] f32 = 2 PSUM banks,
            # exp drains both with one FD=1024 activation.
            h, j = units[k]
            if g == 0:
                pts[k] = ptp.tile([P, 16, SQ], bf16, tag="pt", bufs=2,
                                  name=f"pt{k}")
            stg = psp.tile([P, 2, SQ], f32, tag="stg", bufs=3, name="stg")
            for i in range(2):
                c = 2 * g + i
                nc.tensor.matmul(stg[:, i, :], kT[:, c * P:(c + 1) * P],
                                 qkvT[1 + h][:, j * SQ:(j + 1) * SQ],
                                 start=True, stop=True)
            nc.scalar.activation(pts[k][:, 2 * g:2 * g + 2, :], stg[:], EXP)

        def tree(k):
            pt = pts[k]
            acc8 = sump.tile([P, 8, SQ], bf16, tag="acc8", bufs=1)
            nc.vector.tensor_tensor(acc8[:], pt[:, 0:8, :], pt[:, 8:16, :],
                                    op=ADD)
            acc4 = sump.tile([P, 4, SQ], bf16, tag="acc4", bufs=1)
            nc.vector.tensor_tensor(acc4[:], acc8[:, 0:4, :], acc8[:, 4:8, :],
                                    op=ADD)
            acc2 = sump.tile([P, 2, SQ], bf16, tag="acc2", bufs=1)
            nc.vector.tensor_tensor(acc2[:], acc4[:, 0:2, :], acc4[:, 2:4, :],
                                    op=ADD)
            accs[k] = sump.tile([P, SQ], bf16, tag="accS", bufs=2,
                                name=f"accS{k}")
            nc.vector.tensor_tensor(accs[k][:], acc2[:, 0, :], acc2[:, 1, :],
                                    op=ADD)

        def colsum(k):
            # single ones-matmul over the tree result -> l in lp[0:1, :]
            lps[k] = psp.tile([P, SQ], f32, tag="lp", bufs=1, name="lp")
            nc.tensor.matmul(lps[k][0:1, :], ones128[:], accs[k][:],
                             start=True, stop=True)

        def recip(k):
            # bf16 output so the broadcast matmul is single-pass bf16
            rrs[k] = sump.tile([1, SQ], bf16, tag="rr", bufs=2, name=f"rr{k}")
            with nc.allow_low_precision(reason="1/l in bf16: 0.4% rel, "
                                        "within the softmax error budget"):
                nc.vector.reciprocal(rrs[k][:], lps[k][0:1, :])

        def bcast(k):
            # broadcast 1/l to 128 partitions via K=1 matmul, then to SBUF
            nc.tensor.matmul(lps[k][:], onesk1[:], rrs[k][:],
                             start=True, stop=True)

        def lcopy(k):
            lbs[k] = sump.tile([P, SQ], f32, tag="lb", bufs=2, name=f"lb{k}")
            nc.scalar.copy(lbs[k][:], lps[k][:])

        def av_chunks(k, c0, c1):
            if c0 == 0:
                ops[k] = psp.tile([P, SQ], f32, tag="op", bufs=1, name=f"op{k}")
            for c in range(c0, c1):
                nc.tensor.matmul(ops[k][:], v_sb[:, c, :], pts[k][:, c, :],
                                 start=(c == 0), stop=(c == 15))

        def norm(k):
            h, j = units[k]
            nc.vector.tensor_tensor(ot_sb[:, h, j * SQ:(j + 1) * SQ],
                                    ops[k][:], lbs[k][:], op=MULT)
            del pts[k], ops[k], lbs[k], accs[k], rrs[k], lps[k]

        # Steady-state iteration k (unit u0=k doing ST/exp, u1=k-1 the
        # sums/A@V, u2=k-2 the normalize):
        #   DVE: norm(u2) first (frees the single op bank for AV(u1)),
        #        tree(u1), recip(u1) (after the mid-iter colsum matmul)
        #   PE : ST groups with AV chunks between them, colsum(u1) after
        #        g4 (tree done), bcast(u1) last (recip just finished)
        #   ACT: the 8 exps, then lcopy(u1) (after bcast)
        for k in range(NU + 3):
            u0, u1, u2 = k, k - 1, k - 2
            live0 = u0 < NU
            live1 = 0 <= u1 < NU
            live2 = 0 <= u2 < NU
            if live2:
                norm(u2)
            if live0:
                st_group(u0, 0)
                st_group(u0, 1)
            if live1:
                tree(u1)
            for g in range(2, 8):
                if live1:
                    av_chunks(u1, 2 * (g - 2), 2 * (g - 1))
                if live0:
                    st_group(u0, g)
                if g == 4 and live1:
                    colsum(u1)
                    recip(u1)
                if g == 5:
                    ph3_sub(k)
            if live1:
                av_chunks(u1, 12, 16)
                bcast(u1)
                lcopy(u1)
            ph3_sub(k)

        # remaining output-projection sub-blocks (dense tail)
        while p3[0] < len(p3q):
            ph3_sub(10 ** 9)


def _emit(tc, d):
    nc = tc.nc
    with (
        tc.tile_pool(name="const", bufs=1) as constp,
        tc.tile_pool(name="persist", bufs=1) as pp,
    ):
        ident = constp.tile([P, P], bf16, tag="ident")
        make_identity(nc, ident[:])
        ones128 = constp.tile([P, 1], bf16, tag="ones128")
        nc.gpsimd.memset(ones128[:], 1.0)
        onesk1 = constp.tile([1, P], bf16, tag="onesk1")
        nc.gpsimd.memset(onesk1[:], 1.0)

        qkvT = [pp.tile([P, S], bf16, tag=f"qkv{i}", name=f"qkv{i}")
                for i in range(5)]
        v_sb = pp.tile([P, 16, P], bf16, tag="v")      # V[k,d] in 16 k-chunks
        ot_sb = pp.tile([P, QH, S], bf16, tag="ot")    # O^T per head [d, s]

        _phase1(tc, d, ident, qkvT, v_sb)

        with tc.tile_pool(name="wo", bufs=1) as wop:
            woutt = [wop.tile([P, HIDDEN], bf16, tag=f"wo{c}", name=f"wo{c}")
                     for c in range(QH)]
            for c in range(QH):
                nc.sync.dma_start(out=woutt[c][:],
                                  in_=d["wout"][c * P:(c + 1) * P, :])
            _phase2(tc, d, ones128, onesk1, qkvT, v_sb, ot_sb, woutt)


def _legalize_waits(nc):
    """This toolchain's codegen accepts at most ONE sync wait per
    instruction; hoist extra waits onto single-wait Drain clones inserted
    just before the instruction on the same engine."""
    import copy
    f = nc.m.functions[0]
    templates = {}
    for blk in f.blocks:
        for inst in blk.instructions:
            if type(inst).__name__ == "InstDrain":
                templates.setdefault(str(inst.engine), inst)
    anyt = next(iter(templates.values()))
    SI = type(next(i for b in f.blocks for i in b.instructions
                   if i.sync_info).sync_info)
    k = 0
    for blk in f.blocks:
        newl = []
        for inst in blk.instructions:
            si = inst.sync_info
            if si and len(si.on_wait) > 1:
                for w in si.on_wait:
                    dcl = copy.deepcopy(templates.get(str(inst.engine), anyt))
                    dcl.engine = inst.engine
                    dcl.name = f"{inst.name}w{k}"; k += 1
                    dcl.sync_info = SI(on_wait=[w], on_update=[])
                    newl.append(dcl)
                inst.sync_info = SI(on_wait=[], on_update=list(si.on_update))
            newl.append(inst)
        try:
            blk.instructions[:] = newl
        except Exception:
            blk.instructions = newl


def build():
    nc = bass.Bass()
    d = {
        "xq": nc.dram_tensor("xq", [NQ, P, HCH, SQ], bf16, kind="ExternalInput"),
        "wq": nc.dram_tensor("wq", [AT, P, HCH, P], bf16, kind="ExternalInput"),
        "wout": nc.dram_tensor("wout", [QH * P, HIDDEN], bf16,
                               kind="ExternalInput"),
        "ctab": nc.dram_tensor("ctab", [P, S], f32, kind="ExternalInput"),
        "stab": nc.dram_tensor("stab", [ROT, S], f32, kind="ExternalInput"),
        "perm": nc.dram_tensor("perm", [ROT, ROT], bf16, kind="ExternalInput"),
        "y": nc.dram_tensor("y", [S, HIDDEN], bf16, kind="ExternalOutput"),
    }
    with tile.TileContext(nc) as tc:
        _emit(tc, d)
    _legalize_waits(nc)
    return nc


_NC_CACHE = {}


def _get_nc():
    if "nc" not in _NC_CACHE:
        _NC_CACHE["nc"] = build()
    return _NC_CACHE["nc"]


def make_in_maps(x, W_qkv, W_out):
    xT = np.ascontiguousarray(x.reshape(S, HIDDEN).T)
    # [4096, 2048] -> (c p) (q s) -> [q, p, c, s]
    xq = np.ascontiguousarray(
        xT.reshape(HCH, P, NQ, SQ).transpose(2, 1, 0, 3)).astype(BF16)

    perm = np.concatenate([np.arange(0, ROT, 2), np.arange(1, ROT, 2),
                           np.arange(ROT, P)])
    scale = np.float32(P ** -0.5)

    inv = (10000.0 ** (-np.arange(32) / 32.0)).astype(np.float64)
    fr = np.arange(S, dtype=np.float64)[:, None] * inv[None, :]
    cos = np.cos(fr).T.astype(np.float32)   # [32, S]
    sin = np.sin(fr).T.astype(np.float32)
    ctab = np.concatenate([cos, cos, np.ones((ROT, S), np.float32)], axis=0)
    stab = np.concatenate([-sin, sin], axis=0)
    # swap matrix: swp[m, :] = stage[(m+32) % 64, :]
    permm = np.zeros((ROT, ROT), np.float32)
    for m in range(ROT):
        permm[(m + 32) % ROT, m] = 1.0
    permm = permm.astype(BF16)

    in_maps = []
    for g in range(N_CORES):
        kcol = W_qkv[:, (32 + g) * P:(33 + g) * P][:, perm]
        vcol = W_qkv[:, (40 + g) * P:(41 + g) * P]
        qcols = [W_qkv[:, (4 * g + i) * P:(4 * g + i + 1) * P][:, perm] * scale
                 for i in range(QH)]
        wc = np.stack([kcol, vcol] + qcols, axis=0)          # [6, 4096, 128]
        wq = np.ascontiguousarray(
            wc.reshape(AT, HCH, P, P).transpose(0, 2, 1, 3)).astype(BF16)
        wout = np.ascontiguousarray(
            W_out[g * QH * P:(g + 1) * QH * P, :]).astype(BF16)
        in_maps.append({"xq": xq, "wq": wq, "wout": wout,
                        "ctab": ctab, "stab": stab, "perm": permm})
    return in_maps


def run(x, W_qkv, W_out, trace=False):
    nc = _get_nc()
    in_maps = make_in_maps(np.asarray(x, dtype=np.float32),
                           np.asarray(W_qkv, dtype=np.float32),
                           np.asarray(W_out, dtype=np.float32))
    res = run_bass_kernel_spmd(nc, in_maps, list(range(N_CORES)), trace=trace)
    y = np.zeros((S, HIDDEN), np.float32)
    for om in res.results:
        y += np.asarray(om["y"], dtype=np.float32)
    return y.reshape(1, S, HIDDEN), res


def kernel(x, W_qkv, W_out):
    y, _ = run(x, W_qkv, W_out)
    return y
